# revision 1
# baseline (speedup 1.0000x reference)
"""AttnOutputDecoder Trainium2 kernel.

Sharding: data-parallel over batch B=16 across 8 cores (2 batches/core).
Each core: LSTM (transposed, W-stationary bf16 matmuls) -> Bahdanau
attention (tanh via ACT per-partition bias) -> output proj -> full-vocab
projection (bf16, streamed). Host does embedding gather, transposes,
bf16 casts, and the input projection x @ W_ih.T (not recurrent).
"""

import numpy as np
import ml_dtypes

import concourse.bass as bass
import concourse.mybir as mybir
import concourse.tile as tile
from concourse import bacc
from concourse import bass_utils

BF16 = ml_dtypes.bfloat16
F32 = mybir.dt.float32
BF = mybir.dt.bfloat16
AF = mybir.ActivationFunctionType
ALU = mybir.AluOpType

B, T, S, D, V = 16, 64, 128, 512, 32000
NC = 8
BL = B // NC          # local batches per core = 2
R = BL * T            # local rows = 128
G4 = 4 * D            # 2048 gates
KC = D // 128         # 4 contraction chunks
VBLK = 512

_cached = {}


def _build_nc():
    nc = bacc.Bacc("TRN2", target_bir_lowering=False, debug=False,
                   num_devices=NC)

    def din(name, shape, dt):
        return nc.dram_tensor(name, shape, dt, kind="ExternalInput").ap()

    t_xg = din("xg", [128, 16 * 128], F32)          # [p,(j,t,b)] gate-chunk j
    t_whh = din("whh", [128, KC * G4], BF)           # [p,(kc,g)] = W_hh.T re
    t_h0 = din("h0", [128, KC * BL], F32)            # [p,(kc,b)]
    t_c0 = din("c0", [128, KC * BL], F32)
    t_encT = din("encT", [128, KC * BL * S], BF)     # [p,(kc,b,s)]
    t_enc = din("enc", [128, BL * D], BF)            # [s,(b,d)]
    t_whT = din("whT", [128, KC * D], BF)            # [p,(kc,d)] Wh_w.T re
    t_wsT = din("wsT", [128, KC * D], BF)
    t_vw1 = din("vw1", [128, KC * D], BF)            # (V_w[:,:D]).T re
    t_vw2 = din("vw2", [128, KC * D], BF)
    t_wsb = din("wsb", [128, KC], F32)               # Ws_b chunks
    t_vb = din("vb", [128, KC], F32)                 # V_b chunks
    t_vt = din("vt", [128, KC], BF)                  # vt_w chunks
    t_vpt = din("vpt", [128, KC * V], BF)            # [p,(kc,v)] Vp_w.T re
    t_vpb = din("vpb", [1, V], BF)
    t_ones = din("ones", [1, 128], BF)
    t_ident = din("ident", [128, 128], BF)
    t_out = nc.dram_tensor("out", [R, V], F32, kind="ExternalOutput").ap()

    with tile.TileContext(nc) as tc:
        with (
            tc.tile_pool(name="const", bufs=1) as cp,
            tc.tile_pool(name="state", bufs=1) as sp,
            tc.tile_pool(name="gates", bufs=2) as gp,
            tc.tile_pool(name="attn", bufs=3) as ap_,
            tc.tile_pool(name="voc", bufs=3) as vp,
            tc.tile_pool(name="ps_g", bufs=2, space="PSUM") as ppg,
            tc.tile_pool(name="ps_e", bufs=1, space="PSUM") as ppe,
            tc.tile_pool(name="ps_sm", bufs=2, space="PSUM") as pps,
            tc.tile_pool(name="ps_v", bufs=2, space="PSUM") as ppv,
        ):
            # ---- resident constants ----
            whh = cp.tile([128, KC * G4], BF)
            nc.sync.dma_start(out=whh[:], in_=t_whh[:])
            xg = cp.tile([128, 16 * 128], F32)
            nc.sync.dma_start(out=xg[:], in_=t_xg[:])
            encT = cp.tile([128, KC * BL * S], BF)
            nc.sync.dma_start(out=encT[:], in_=t_encT[:])
            enc = cp.tile([128, BL * D], BF)
            nc.sync.dma_start(out=enc[:], in_=t_enc[:])
            whT = cp.tile([128, KC * D], BF)
            nc.sync.dma_start(out=whT[:], in_=t_whT[:])
            wsT = cp.tile([128, KC * D], BF)
            nc.sync.dma_start(out=wsT[:], in_=t_wsT[:])
            vw1 = cp.tile([128, KC * D], BF)
            nc.sync.dma_start(out=vw1[:], in_=t_vw1[:])
            vw2 = cp.tile([128, KC * D], BF)
            nc.sync.dma_start(out=vw2[:], in_=t_vw2[:])
            wsb = cp.tile([128, KC], F32)
            nc.sync.dma_start(out=wsb[:], in_=t_wsb[:])
            vb = cp.tile([128, KC], F32)
            nc.sync.dma_start(out=vb[:], in_=t_vb[:])
            vt = cp.tile([128, KC], BF)
            nc.sync.dma_start(out=vt[:], in_=t_vt[:])
            ones = cp.tile([1, 128], BF)
            nc.sync.dma_start(out=ones[:], in_=t_ones[:])
            ident = cp.tile([128, 128], BF)
            nc.sync.dma_start(out=ident[:], in_=t_ident[:])

            # ---- state ----
            h = sp.tile([128, KC * BL], F32)    # h_T [p,(kc,b)]
            c = sp.tile([128, KC * BL], F32)
            nc.sync.dma_start(out=h[:], in_=t_h0[:])
            nc.sync.dma_start(out=c[:], in_=t_c0[:])
            hbf = sp.tile([128, KC * BL], BF)
            nc.vector.tensor_copy(out=hbf[:], in_=h[:])
            outT = sp.tile([128, KC * BL * T], BF)   # [p,(kc,b,t)] all h's

            xg4 = xg[:].rearrange("p (j t b) -> p j t b", j=16, t=T, b=BL)
            outT4 = outT[:].rearrange("p (kc b t) -> p kc b t", kc=KC, b=BL,
                                      t=T)

            # ---- vocab weight prefetch (hidden under compute) ----
            NPRE = 32
            vpt4 = t_vpt[:].rearrange("p (kc v) -> p kc v", kc=KC, v=V)
            vpre = cp.tile([128, NPRE * KC * VBLK], BF)
            vpre4 = vpre[:].rearrange("p (i kc v) -> p i kc v", i=NPRE,
                                      kc=KC, v=VBLK)
            for i in range(NPRE):
                for kc in range(KC):
                    nc.sync.dma_start(out=vpre4[:, i, kc, :],
                                      in_=vpt4[:, kc, i * VBLK:(i + 1) * VBLK])

            # ====== wh = enc @ Wh_w.T  (before LSTM; -> sbuf bf16) ======
            whs = sp.tile([128, BL * KC * 128], BF)   # [p,(b,dc,s)]
            for b in range(BL):
                whp = ppv.tile([128, VBLK], F32, tag="lps")
                for dc in range(KC):
                    for kc in range(KC):
                        nc.tensor.matmul(
                            out=whp[:, dc * 128:(dc + 1) * 128],
                            lhsT=whT[:, kc * D + dc * 128: kc * D + (dc + 1) * 128],
                            rhs=encT[:, (kc * BL + b) * S:(kc * BL + b + 1) * S],
                            start=(kc == 0), stop=(kc == KC - 1))
                nc.vector.tensor_copy(out=whs[:, b * 512:(b + 1) * 512],
                                      in_=whp[:])

            wst = sp.tile([128, KC * BL * T], F32)   # [p,(dc,b,t)]
            eps0 = ppe.tile([S, T], F32, tag="e0")
            eps1 = ppe.tile([S, T], F32, tag="e1")
            epss = [eps0, eps1]

            # ========== LSTM + blocked attention-score overlap ==========
            def emit_score(b, t):
                for dc in range(KC):
                    th = ap_.tile([128, S], BF, tag="th", name=f"th{b}_{t}_{dc}")
                    nc.scalar.activation(
                        out=th[:],
                        in_=whs[:, b * 512 + dc * 128:
                                b * 512 + (dc + 1) * 128],
                        func=AF.Tanh,
                        bias=wst[:, (dc * BL + b) * T + t:
                                 (dc * BL + b) * T + t + 1])
                    nc.tensor.matmul(out=epss[b][:, t:t + 1],
                                     lhsT=th[:], rhs=vt[:, dc:dc + 1],
                                     start=(dc == 0), stop=(dc == KC - 1))

            pending = []
            TB = 16
            for blk in range(T // TB):
                tlo = blk * TB
                for t in range(tlo, tlo + TB):
                    gps = ppg.tile([128, 16 * BL], F32, tag="gps")
                    for j in range(16):
                        for kc in range(KC):
                            nc.tensor.matmul(
                                out=gps[:, j * BL:(j + 1) * BL],
                                lhsT=whh[:, kc * G4 + j * 128:
                                          kc * G4 + (j + 1) * 128],
                                rhs=hbf[:, kc * BL:(kc + 1) * BL],
                                start=(kc == 0), stop=(kc == KC - 1))
                    gs = gp.tile([128, 16 * BL], F32, tag="gs")
                    gps3 = gps[:].rearrange("p (j b) -> p j b", j=16, b=BL)
                    gs3 = gs[:].rearrange("p (j b) -> p j b", j=16, b=BL)
                    nc.vector.tensor_add(out=gs3, in0=gps3, in1=xg4[:, :, t, :])
                    sio = gp.tile([128, 16 * BL], F32, tag="sio")
                    nc.scalar.activation(out=sio[:, 0:8 * BL],
                                         in_=gs[:, 0:8 * BL], func=AF.Sigmoid)
                    nc.scalar.activation(out=sio[:, 12 * BL:16 * BL],
                                         in_=gs[:, 12 * BL:16 * BL],
                                         func=AF.Sigmoid)
                    nc.scalar.activation(out=sio[:, 8 * BL:12 * BL],
                                         in_=gs[:, 8 * BL:12 * BL],
                                         func=AF.Tanh)
                    t1 = gp.tile([128, KC * BL], F32, tag="t1")
                    t2 = gp.tile([128, KC * BL], F32, tag="t2")
                    nc.vector.tensor_mul(out=t1[:], in0=sio[:, 4 * BL:8 * BL],
                                         in1=c[:])
                    nc.vector.tensor_mul(out=t2[:], in0=sio[:, 0:4 * BL],
                                         in1=sio[:, 8 * BL:12 * BL])
                    nc.vector.tensor_add(out=c[:], in0=t1[:], in1=t2[:])
                    tc_ = gp.tile([128, KC * BL], F32, tag="tc")
                    nc.scalar.activation(out=tc_[:], in_=c[:], func=AF.Tanh)
                    nc.vector.tensor_mul(out=h[:],
                                         in0=sio[:, 12 * BL:16 * BL],
                                         in1=tc_[:])
                    nc.vector.tensor_copy(out=hbf[:], in_=h[:])
                    hbf3 = hbf[:].rearrange("p (kc b) -> p kc b", kc=KC, b=BL)
                    nc.vector.tensor_copy(out=outT4[:, :, :, t], in_=hbf3)
                    for _ in range(min(8, len(pending))):
                        emit_score(*pending.pop(0))

                # ws for this t-block
                for b in range(BL):
                    for dc in range(KC):
                        wps = pps.tile([128, TB], F32, tag="sm")
                        for kc in range(KC):
                            nc.tensor.matmul(
                                out=wps[:],
                                lhsT=wsT[:, kc * D + dc * 128:
                                         kc * D + (dc + 1) * 128],
                                rhs=outT[:, (kc * BL + b) * T + tlo:
                                         (kc * BL + b) * T + tlo + TB],
                                start=(kc == 0), stop=(kc == KC - 1))
                        nc.vector.tensor_scalar(
                            out=wst[:, (dc * BL + b) * T + tlo:
                                    (dc * BL + b) * T + tlo + TB],
                            in0=wps[:], scalar1=wsb[:, dc:dc + 1],
                            scalar2=None, op0=ALU.add)

                # queue this block's score tasks; emitted interleaved
                # with the next block's LSTM steps (keeps ACT round-robin)
                pending.extend((b, t) for b in range(BL)
                               for t in range(tlo, tlo + TB))

            # ============ scores, softmax, context, out2 ============
            ctxT = sp.tile([128, BL * KC * T], BF)   # [p,(b,dc,t)]
            o2T = sp.tile([128, KC * BL * T], BF)    # [p,(ec,b,t)]
            while pending:
                emit_score(*pending.pop(0))

            for b in range(BL):
                eps = epss[b]
                # softmax over s; |e| is small so no max-subtract needed
                ebf = ap_.tile([S, T], BF, tag="ebf")
                nc.scalar.activation(out=ebf[:], in_=eps[:], func=AF.Exp)
                # transpose exp(e).T -> [t, s]
                etp = pps.tile([T, S], BF, tag="sm")
                nc.tensor.transpose(out=etp[:], in_=ebf[:],
                                    identity=ident[:, :])
                ssum = ap_.tile([T, 1], F32, tag="ssum")
                nc.vector.tensor_reduce(out=ssum[:], in_=etp[:],
                                        axis=mybir.AxisListType.X, op=ALU.add)
                rsum = ap_.tile([T, 1], F32, tag="rsum")
                nc.vector.reciprocal(out=rsum[:], in_=ssum[:])
                abf = ap_.tile([T, S], BF, tag="abf")
                nc.vector.tensor_scalar_mul(out=abf[:], in0=etp[:],
                                            scalar1=rsum[:])
                # transpose a -> [s, t]
                atp = pps.tile([S, T], BF, tag="sm")
                nc.tensor.transpose(out=atp[:], in_=abf[:],
                                    identity=ident[0:T, 0:T])
                atb = ap_.tile([S, T], BF, tag="atb")
                nc.vector.tensor_copy(out=atb[:], in_=atp[:])
                # context: ctxT[d,t] = enc.T @ a
                for dc in range(KC):
                    cps = pps.tile([128, T], F32, tag="sm")
                    nc.tensor.matmul(out=cps[:],
                                     lhsT=enc[:, b * D + dc * 128:
                                              b * D + (dc + 1) * 128],
                                     rhs=atb[:], start=True, stop=True)
                    nc.vector.tensor_copy(
                        out=ctxT[:, (b * KC + dc) * T:(b * KC + dc + 1) * T],
                        in_=cps[:])
                # out2 = [ctx|out] @ V_w.T + V_b   (transposed)
                for ec in range(KC):
                    ops = pps.tile([128, T], F32, tag="sm")
                    for kc in range(KC):
                        nc.tensor.matmul(
                            out=ops[:],
                            lhsT=vw1[:, kc * D + ec * 128: kc * D + (ec + 1) * 128],
                            rhs=ctxT[:, (b * KC + kc) * T:(b * KC + kc + 1) * T],
                            start=(kc == 0), stop=False)
                    for kc in range(KC):
                        nc.tensor.matmul(
                            out=ops[:],
                            lhsT=vw2[:, kc * D + ec * 128: kc * D + (ec + 1) * 128],
                            rhs=outT[:, (kc * BL + b) * T:(kc * BL + b + 1) * T],
                            start=False, stop=(kc == KC - 1))
                    nc.vector.tensor_scalar(
                        out=o2T[:, (ec * BL + b) * T:(ec * BL + b + 1) * T],
                        in0=ops[:], scalar1=vb[:, ec:ec + 1], scalar2=None,
                        op0=ALU.add)

            # ================= vocab projection =================
            for ib, v0 in enumerate(range(0, V, VBLK)):
                w = min(VBLK, V - v0)
                if ib < NPRE:
                    vsrc = vpre4[:, ib]
                else:
                    vps = vp.tile([128, KC, VBLK], BF, tag="vps")
                    for kc in range(KC):
                        nc.sync.dma_start(out=vps[:, kc, :w],
                                          in_=vpt4[:, kc, v0:v0 + w])
                    vsrc = vps
                vpbt = vp.tile([1, VBLK], BF, tag="vpbt")
                nc.sync.dma_start(out=vpbt[:, :w], in_=t_vpb[:, v0:v0 + w])
                lps = ppv.tile([128, VBLK], F32, tag="lps")
                for kc in range(KC):
                    nc.tensor.matmul(out=lps[:, :w],
                                     lhsT=o2T[:, kc * 128:(kc + 1) * 128],
                                     rhs=vsrc[:, kc, :w],
                                     start=(kc == 0), stop=False)
                nc.tensor.matmul(out=lps[:, :w], lhsT=ones[:],
                                 rhs=vpbt[:, :w], start=False, stop=True)
                lsb = vp.tile([128, VBLK], F32, tag="lsb")
                if ib % 2 == 0:
                    nc.scalar.copy(out=lsb[:, :w], in_=lps[:, :w])
                else:
                    nc.vector.tensor_copy(out=lsb[:, :w], in_=lps[:, :w])
                nc.sync.dma_start(out=t_out[:, v0:v0 + w], in_=lsb[:, :w])

    nc.compile()
    return nc


def _prep_in_maps(inputs):
    inp = {k: np.asarray(v) for k, v in inputs.items()}
    words = inp["words"].astype(np.int64)
    enc = inp["encoder_output"].astype(np.float32)
    pre_h, cell = inp["pre_h"], inp["cell"]
    emb = inp["emb"]
    W_ih, W_hh = inp["W_ih"], inp["W_hh"]
    b_ih, b_hh = inp["b_ih"], inp["b_hh"]
    Wh_w = inp["Wh_w"]
    Ws_w, Ws_b = inp["Ws_w"], inp["Ws_b"]
    vt_w = inp["vt_w"]
    V_w, V_b = inp["V_w"], inp["V_b"]
    Vp_w, Vp_b = inp["Vp_w"], inp["Vp_b"]

    def re_lhsT(m):  # [512, N] -> [128, 4*N] chunk-major, bf16
        n = m.shape[1]
        return np.ascontiguousarray(
            m.reshape(4, 128, n).transpose(1, 0, 2).reshape(128, 4 * n)
        ).astype(BF16)

    whh_re = re_lhsT(np.ascontiguousarray(W_hh.T))
    whT_re = re_lhsT(np.ascontiguousarray(Wh_w.T))
    wsT_re = re_lhsT(np.ascontiguousarray(Ws_w.T))
    vw1_re = re_lhsT(np.ascontiguousarray(V_w[:, :D].T))
    vw2_re = re_lhsT(np.ascontiguousarray(V_w[:, D:].T))
    vpt_re = re_lhsT(np.ascontiguousarray(Vp_w.T))
    wsb_re = np.ascontiguousarray(Ws_b.reshape(4, 128).T).astype(np.float32)
    vb_re = np.ascontiguousarray(V_b.reshape(4, 128).T).astype(np.float32)
    vt_re = np.ascontiguousarray(vt_w.reshape(4, 128).T).astype(BF16)
    vpb_re = Vp_b.reshape(1, V).astype(BF16)
    ones_re = np.ones((1, 128), dtype=BF16)
    ident_re = np.eye(128, dtype=np.float32).astype(BF16)

    bias2 = (b_ih + b_hh).astype(np.float32)
    x_all = emb[words]                                   # [B,T,D]
    xg_all = x_all @ W_ih.T.astype(np.float32) + bias2   # [B,T,4D]

    in_maps = []
    for k in range(NC):
        bs = slice(k * BL, (k + 1) * BL)
        xg = xg_all[bs]                                  # [2,T,2048]
        xg_re = np.ascontiguousarray(
            xg.reshape(BL, T, 16, 128).transpose(3, 2, 1, 0)
            .reshape(128, 16 * T * BL)).astype(np.float32)
        h0 = np.ascontiguousarray(
            pre_h[bs].reshape(BL, 4, 128).transpose(2, 1, 0)
            .reshape(128, 4 * BL)).astype(np.float32)
        c0 = np.ascontiguousarray(
            cell[bs].reshape(BL, 4, 128).transpose(2, 1, 0)
            .reshape(128, 4 * BL)).astype(np.float32)
        encl = enc[bs]                                   # [2,S,D]
        encT_re = np.ascontiguousarray(
            encl.reshape(BL, S, 4, 128).transpose(3, 2, 0, 1)
            .reshape(128, 4 * BL * S)).astype(BF16)
        enc_re = np.ascontiguousarray(
            encl.transpose(1, 0, 2).reshape(S, BL * D)).astype(BF16)
        in_maps.append({
            "xg": xg_re, "whh": whh_re, "h0": h0, "c0": c0,
            "encT": encT_re, "enc": enc_re, "whT": whT_re, "wsT": wsT_re,
            "vw1": vw1_re, "vw2": vw2_re, "wsb": wsb_re, "vb": vb_re,
            "vt": vt_re, "vpt": vpt_re, "vpb": vpb_re, "ones": ones_re,
            "ident": ident_re,
        })
    return in_maps


def kernel(**inputs):
    in_maps = _prep_in_maps(inputs)
    if "nc" not in _cached:
        _cached["nc"] = _build_nc()
    res = bass_utils.run_bass_kernel_spmd(_cached["nc"], in_maps,
                                          core_ids=list(range(NC)))
    outs = [res.results[k]["out"].reshape(BL, T, V) for k in range(NC)]
    return np.concatenate(outs, axis=0).astype(np.float32)


if __name__ == "__main__":
    pass



# revision 9
# speedup vs baseline: 1.0616x; 1.0616x over previous
"""AttnOutputDecoder Trainium2 kernel.

Sharding: data-parallel over batch B=16 across 8 cores (2 batches/core).
Each core: LSTM (transposed, W-stationary bf16 matmuls) -> Bahdanau
attention (tanh via ACT per-partition bias) -> output proj -> full-vocab
projection (bf16, streamed). Host does embedding gather, transposes,
bf16 casts, and the input projection x @ W_ih.T (not recurrent).
"""

import numpy as np
import ml_dtypes

import concourse.bass as bass
import concourse.mybir as mybir
import concourse.tile as tile
from concourse import bacc
from concourse import bass_utils

BF16 = ml_dtypes.bfloat16
F32 = mybir.dt.float32
BF = mybir.dt.bfloat16
AF = mybir.ActivationFunctionType
ALU = mybir.AluOpType

B, T, S, D, V = 16, 64, 128, 512, 32000
NC = 8
BL = B // NC          # local batches per core = 2
R = BL * T            # local rows = 128
G4 = 4 * D            # 2048 gates
KC = D // 128         # 4 contraction chunks
VB = 2000             # vocab block (one DMA; 4 matmul sub-blocks of 500)
NVB = V // VB         # 16 vocab blocks
VSUB = 500            # matmul moving-dim sub-block
NPREB = 6             # vocab blocks prefetched resident in SBUF

_cached = {}


def _build_nc():
    nc = bacc.Bacc("TRN2", target_bir_lowering=False, debug=False,
                   num_devices=NC)

    def din(name, shape, dt):
        return nc.dram_tensor(name, shape, dt, kind="ExternalInput").ap()

    t_xg = din("xg", [128, 16 * 128], F32)          # [p,(j,t,b)] gate-chunk j
    t_whh = din("whh", [128, KC * G4], BF)           # [p,(kc,g)] = W_hh.T re
    t_h0 = din("h0", [128, KC * BL], F32)            # [p,(kc,b)]
    t_c0 = din("c0", [128, KC * BL], F32)
    t_encT = din("encT", [128, KC * BL * S], BF)     # [p,(kc,b,s)]
    t_enc = din("enc", [128, BL * D], BF)            # [s,(b,d)]
    t_whT = din("whT", [128, KC * D], BF)            # [p,(kc,d)] Wh_w.T re
    t_wsT = din("wsT", [128, KC * D], BF)
    t_vw1 = din("vw1", [128, KC * D], BF)            # (V_w[:,:D]).T re
    t_vw2 = din("vw2", [128, KC * D], BF)
    t_wsb = din("wsb", [128, KC], F32)               # Ws_b chunks
    t_vb = din("vb", [128, KC], F32)                 # V_b chunks
    t_vt = din("vt", [128, KC], BF)                  # vt_w chunks
    t_vpt = din("vpt", [128, NVB * KC * VB], BF)     # [p,(nb,kc,v)] Vp_w.T re
    t_ones = din("ones", [1, 128], BF)
    t_ident = din("ident", [128, 128], BF)
    t_out = nc.dram_tensor("out", [R, V], BF, kind="ExternalOutput").ap()

    with tile.TileContext(nc) as tc:
        with (
            tc.tile_pool(name="const", bufs=1) as cp,
            tc.tile_pool(name="state", bufs=1) as sp,
            tc.tile_pool(name="gates", bufs=2) as gp,
            tc.tile_pool(name="attn", bufs=3) as ap_,
            tc.tile_pool(name="voc", bufs=3) as vp,
            tc.tile_pool(name="ps_g", bufs=2, space="PSUM") as ppg,
            tc.tile_pool(name="ps_e", bufs=1, space="PSUM") as ppe,
            tc.tile_pool(name="ps_sm", bufs=2, space="PSUM") as pps,
            tc.tile_pool(name="ps_v", bufs=2, space="PSUM") as ppv,
        ):
            # ---- resident constants ----
            whh = cp.tile([128, KC * G4], BF)
            nc.sync.dma_start(out=whh[:], in_=t_whh[:])
            xg = cp.tile([128, 16 * 128], F32)
            nc.sync.dma_start(out=xg[:], in_=t_xg[:])
            encT = cp.tile([128, KC * BL * S], BF)
            nc.sync.dma_start(out=encT[:], in_=t_encT[:])
            enc = cp.tile([128, BL * D], BF)
            nc.sync.dma_start(out=enc[:], in_=t_enc[:])
            whT = cp.tile([128, KC * D], BF)
            nc.sync.dma_start(out=whT[:], in_=t_whT[:])
            wsT = cp.tile([128, KC * D], BF)
            nc.sync.dma_start(out=wsT[:], in_=t_wsT[:])
            vw1 = cp.tile([128, KC * D], BF)
            nc.sync.dma_start(out=vw1[:], in_=t_vw1[:])
            vw2 = cp.tile([128, KC * D], BF)
            nc.sync.dma_start(out=vw2[:], in_=t_vw2[:])
            wsb = cp.tile([128, KC], F32)
            nc.sync.dma_start(out=wsb[:], in_=t_wsb[:])
            vb = cp.tile([128, KC], F32)
            nc.sync.dma_start(out=vb[:], in_=t_vb[:])
            vt = cp.tile([128, KC], BF)
            nc.sync.dma_start(out=vt[:], in_=t_vt[:])
            ones = cp.tile([1, 128], BF)
            nc.sync.dma_start(out=ones[:], in_=t_ones[:])
            ident = cp.tile([128, 128], BF)
            nc.sync.dma_start(out=ident[:], in_=t_ident[:])

            # ---- state ----
            h = sp.tile([128, KC * BL], F32)    # h_T [p,(kc,b)]
            c = sp.tile([128, KC * BL], F32)
            nc.sync.dma_start(out=h[:], in_=t_h0[:])
            nc.sync.dma_start(out=c[:], in_=t_c0[:])
            hbf = sp.tile([128, KC * BL], BF)
            nc.vector.tensor_copy(out=hbf[:], in_=h[:])
            outT = sp.tile([128, KC * BL * T], BF)   # [p,(kc,b,t)] all h's

            xg4 = xg[:].rearrange("p (j t b) -> p j t b", j=16, t=T, b=BL)
            outT4 = outT[:].rearrange("p (kc b t) -> p kc b t", kc=KC, b=BL,
                                      t=T)

            # ---- vocab weight prefetch (hidden under compute) ----
            # one big SWDGE (Pool-issued) DMA per vocab block: 16KB/partition
            vpt4 = t_vpt[:].rearrange("p (nb kc v) -> p nb kc v", nb=NVB,
                                      kc=KC, v=VB)
            vpre = cp.tile([128, NPREB * KC * VB], BF)
            vpre4 = vpre[:].rearrange("p (i kc v) -> p i kc v", i=NPREB,
                                      kc=KC, v=VB)
            for i in range(NPREB):
                nc.gpsimd.dma_start(out=vpre4[:, i], in_=vpt4[:, i])

            # ====== wh = enc @ Wh_w.T  (before LSTM; -> sbuf bf16) ======
            whs = sp.tile([128, BL * KC * 128], BF)   # [p,(b,dc,s)]
            for b in range(BL):
                whp = ppv.tile([128, 512], F32, tag="lps")
                for dc in range(KC):
                    for kc in range(KC):
                        nc.tensor.matmul(
                            out=whp[:, dc * 128:(dc + 1) * 128],
                            lhsT=whT[:, kc * D + dc * 128: kc * D + (dc + 1) * 128],
                            rhs=encT[:, (kc * BL + b) * S:(kc * BL + b + 1) * S],
                            start=(kc == 0), stop=(kc == KC - 1))
                nc.vector.tensor_copy(out=whs[:, b * 512:(b + 1) * 512],
                                      in_=whp[:])

            wst = sp.tile([128, KC * BL * T], F32)   # [p,(dc,b,t)]
            eps0 = ppe.tile([S, T], F32, tag="e0")
            eps1 = ppe.tile([S, T], F32, tag="e1")
            epss = [eps0, eps1]

            # ========== LSTM + blocked attention-score overlap ==========
            def emit_score(b, t):
                for dc in range(KC):
                    th = ap_.tile([128, S], BF, tag="th", name=f"th{b}_{t}_{dc}")
                    nc.scalar.activation(
                        out=th[:],
                        in_=whs[:, b * 512 + dc * 128:
                                b * 512 + (dc + 1) * 128],
                        func=AF.Tanh,
                        bias=wst[:, (dc * BL + b) * T + t:
                                 (dc * BL + b) * T + t + 1])
                    nc.tensor.matmul(out=epss[b][:, t:t + 1],
                                     lhsT=th[:], rhs=vt[:, dc:dc + 1],
                                     start=(dc == 0), stop=(dc == KC - 1))

            pending = []
            TB = 16
            for blk in range(T // TB):
                tlo = blk * TB
                for t in range(tlo, tlo + TB):
                    gps = ppg.tile([128, 16 * BL], F32, tag="gps")
                    for j in range(16):
                        for kc in range(KC):
                            nc.tensor.matmul(
                                out=gps[:, j * BL:(j + 1) * BL],
                                lhsT=whh[:, kc * G4 + j * 128:
                                          kc * G4 + (j + 1) * 128],
                                rhs=hbf[:, kc * BL:(kc + 1) * BL],
                                start=(kc == 0), stop=(kc == KC - 1))
                    gs = gp.tile([128, 16 * BL], F32, tag="gs")
                    gps3 = gps[:].rearrange("p (j b) -> p j b", j=16, b=BL)
                    gs3 = gs[:].rearrange("p (j b) -> p j b", j=16, b=BL)
                    nc.vector.tensor_add(out=gs3, in0=gps3, in1=xg4[:, :, t, :])
                    sio = gp.tile([128, 16 * BL], F32, tag="sio")
                    nc.scalar.activation(out=sio[:, 0:8 * BL],
                                         in_=gs[:, 0:8 * BL], func=AF.Sigmoid)
                    nc.scalar.activation(out=sio[:, 12 * BL:16 * BL],
                                         in_=gs[:, 12 * BL:16 * BL],
                                         func=AF.Sigmoid)
                    nc.scalar.activation(out=sio[:, 8 * BL:12 * BL],
                                         in_=gs[:, 8 * BL:12 * BL],
                                         func=AF.Tanh)
                    t1 = gp.tile([128, KC * BL], F32, tag="t1")
                    t2 = gp.tile([128, KC * BL], F32, tag="t2")
                    nc.vector.tensor_mul(out=t1[:], in0=sio[:, 4 * BL:8 * BL],
                                         in1=c[:])
                    nc.vector.tensor_mul(out=t2[:], in0=sio[:, 0:4 * BL],
                                         in1=sio[:, 8 * BL:12 * BL])
                    nc.vector.tensor_add(out=c[:], in0=t1[:], in1=t2[:])
                    tc_ = gp.tile([128, KC * BL], F32, tag="tc")
                    nc.scalar.activation(out=tc_[:], in_=c[:], func=AF.Tanh)
                    nc.vector.tensor_mul(out=h[:],
                                         in0=sio[:, 12 * BL:16 * BL],
                                         in1=tc_[:])
                    nc.vector.tensor_copy(out=hbf[:], in_=h[:])
                    hbf3 = hbf[:].rearrange("p (kc b) -> p kc b", kc=KC, b=BL)
                    nc.vector.tensor_copy(out=outT4[:, :, :, t], in_=hbf3)
                    for _ in range(min(8, len(pending))):
                        emit_score(*pending.pop(0))

                # ws for this t-block
                for b in range(BL):
                    for dc in range(KC):
                        wps = pps.tile([128, TB], F32, tag="sm")
                        for kc in range(KC):
                            nc.tensor.matmul(
                                out=wps[:],
                                lhsT=wsT[:, kc * D + dc * 128:
                                         kc * D + (dc + 1) * 128],
                                rhs=outT[:, (kc * BL + b) * T + tlo:
                                         (kc * BL + b) * T + tlo + TB],
                                start=(kc == 0), stop=(kc == KC - 1))
                        nc.vector.tensor_scalar(
                            out=wst[:, (dc * BL + b) * T + tlo:
                                    (dc * BL + b) * T + tlo + TB],
                            in0=wps[:], scalar1=wsb[:, dc:dc + 1],
                            scalar2=None, op0=ALU.add)

                # queue this block's score tasks; emitted interleaved
                # with the next block's LSTM steps (keeps ACT round-robin)
                pending.extend((b, t) for b in range(BL)
                               for t in range(tlo, tlo + TB))

            # ============ scores, softmax, context, out2 ============
            ctxT = sp.tile([128, BL * KC * T], BF)   # [p,(b,dc,t)]
            o2T = sp.tile([128, KC * BL * T], BF)    # [p,(ec,b,t)]
            while pending:
                emit_score(*pending.pop(0))

            for b in range(BL):
                eps = epss[b]
                # softmax over s; |e| is small so no max-subtract needed
                ebf = ap_.tile([S, T], BF, tag="ebf")
                nc.scalar.activation(out=ebf[:], in_=eps[:], func=AF.Exp)
                # transpose exp(e).T -> [t, s]
                etp = pps.tile([T, S], BF, tag="sm")
                nc.tensor.transpose(out=etp[:], in_=ebf[:],
                                    identity=ident[:, :])
                ssum = ap_.tile([T, 1], F32, tag="ssum")
                nc.vector.tensor_reduce(out=ssum[:], in_=etp[:],
                                        axis=mybir.AxisListType.X, op=ALU.add)
                rsum = ap_.tile([T, 1], F32, tag="rsum")
                nc.vector.reciprocal(out=rsum[:], in_=ssum[:])
                abf = ap_.tile([T, S], BF, tag="abf")
                nc.vector.tensor_scalar_mul(out=abf[:], in0=etp[:],
                                            scalar1=rsum[:])
                # transpose a -> [s, t]
                atp = pps.tile([S, T], BF, tag="sm")
                nc.tensor.transpose(out=atp[:], in_=abf[:],
                                    identity=ident[0:T, 0:T])
                atb = ap_.tile([S, T], BF, tag="atb")
                nc.vector.tensor_copy(out=atb[:], in_=atp[:])
                # context: ctxT[d,t] = enc.T @ a
                for dc in range(KC):
                    cps = pps.tile([128, T], F32, tag="sm")
                    nc.tensor.matmul(out=cps[:],
                                     lhsT=enc[:, b * D + dc * 128:
                                              b * D + (dc + 1) * 128],
                                     rhs=atb[:], start=True, stop=True)
                    nc.vector.tensor_copy(
                        out=ctxT[:, (b * KC + dc) * T:(b * KC + dc + 1) * T],
                        in_=cps[:])
                # out2 = [ctx|out] @ V_w.T + V_b   (transposed)
                for ec in range(KC):
                    ops = pps.tile([128, T], F32, tag="sm")
                    for kc in range(KC):
                        nc.tensor.matmul(
                            out=ops[:],
                            lhsT=vw1[:, kc * D + ec * 128: kc * D + (ec + 1) * 128],
                            rhs=ctxT[:, (b * KC + kc) * T:(b * KC + kc + 1) * T],
                            start=(kc == 0), stop=False)
                    for kc in range(KC):
                        nc.tensor.matmul(
                            out=ops[:],
                            lhsT=vw2[:, kc * D + ec * 128: kc * D + (ec + 1) * 128],
                            rhs=outT[:, (kc * BL + b) * T:(kc * BL + b + 1) * T],
                            start=False, stop=(kc == KC - 1))
                    nc.vector.tensor_scalar(
                        out=o2T[:, (ec * BL + b) * T:(ec * BL + b + 1) * T],
                        in0=ops[:], scalar1=vb[:, ec:ec + 1], scalar2=None,
                        op0=ALU.add)

            # ================= vocab projection =================
            # Vp_b is added on the host; output is bf16 (upcast on host).
            for ib in range(NVB):
                v0 = ib * VB
                if ib < NPREB:
                    vsrc = vpre4[:, ib]
                else:
                    vps = vp.tile([128, KC, VB], BF, tag="vps")
                    nc.gpsimd.dma_start(out=vps[:], in_=vpt4[:, ib])
                    vsrc = vps
                lsb = vp.tile([128, VB], BF, tag="lsb")
                for sb in range(VB // VSUB):
                    s0 = sb * VSUB
                    lps = ppv.tile([128, VSUB], F32, tag="lps")
                    for kc in range(KC):
                        nc.tensor.matmul(out=lps[:],
                                         lhsT=o2T[:, kc * 128:(kc + 1) * 128],
                                         rhs=vsrc[:, kc, s0:s0 + VSUB],
                                         start=(kc == 0), stop=(kc == KC - 1))
                    if sb % 2 == 0:
                        nc.scalar.copy(out=lsb[:, s0:s0 + VSUB], in_=lps[:])
                    else:
                        nc.vector.tensor_copy(out=lsb[:, s0:s0 + VSUB],
                                              in_=lps[:])
                nc.gpsimd.dma_start(out=t_out[:, v0:v0 + VB], in_=lsb[:])

    nc.compile()
    return nc


def _prep_in_maps(inputs):
    inp = {k: np.asarray(v) for k, v in inputs.items()}
    words = inp["words"].astype(np.int64)
    enc = inp["encoder_output"].astype(np.float32)
    pre_h, cell = inp["pre_h"], inp["cell"]
    emb = inp["emb"]
    W_ih, W_hh = inp["W_ih"], inp["W_hh"]
    b_ih, b_hh = inp["b_ih"], inp["b_hh"]
    Wh_w = inp["Wh_w"]
    Ws_w, Ws_b = inp["Ws_w"], inp["Ws_b"]
    vt_w = inp["vt_w"]
    V_w, V_b = inp["V_w"], inp["V_b"]
    Vp_w, Vp_b = inp["Vp_w"], inp["Vp_b"]

    def re_lhsT(m):  # [512, N] -> [128, 4*N] chunk-major, bf16
        n = m.shape[1]
        return np.ascontiguousarray(
            m.reshape(4, 128, n).transpose(1, 0, 2).reshape(128, 4 * n)
        ).astype(BF16)

    whh_re = re_lhsT(np.ascontiguousarray(W_hh.T))
    whT_re = re_lhsT(np.ascontiguousarray(Wh_w.T))
    wsT_re = re_lhsT(np.ascontiguousarray(Ws_w.T))
    vw1_re = re_lhsT(np.ascontiguousarray(V_w[:, :D].T))
    vw2_re = re_lhsT(np.ascontiguousarray(V_w[:, D:].T))
    # [128,(kc,v)] -> [128,(nb,kc,vb)] so each vocab block is one contiguous
    # 16KB-per-partition DMA
    vpt_re = np.ascontiguousarray(
        re_lhsT(np.ascontiguousarray(Vp_w.T))
        .reshape(128, KC, NVB, VB).transpose(0, 2, 1, 3)
        .reshape(128, NVB * KC * VB))
    wsb_re = np.ascontiguousarray(Ws_b.reshape(4, 128).T).astype(np.float32)
    vb_re = np.ascontiguousarray(V_b.reshape(4, 128).T).astype(np.float32)
    vt_re = np.ascontiguousarray(vt_w.reshape(4, 128).T).astype(BF16)
    ones_re = np.ones((1, 128), dtype=BF16)
    ident_re = np.eye(128, dtype=np.float32).astype(BF16)

    bias2 = (b_ih + b_hh).astype(np.float32)
    x_all = emb[words]                                   # [B,T,D]
    xg_all = x_all @ W_ih.T.astype(np.float32) + bias2   # [B,T,4D]

    in_maps = []
    for k in range(NC):
        bs = slice(k * BL, (k + 1) * BL)
        xg = xg_all[bs]                                  # [2,T,2048]
        xg_re = np.ascontiguousarray(
            xg.reshape(BL, T, 16, 128).transpose(3, 2, 1, 0)
            .reshape(128, 16 * T * BL)).astype(np.float32)
        h0 = np.ascontiguousarray(
            pre_h[bs].reshape(BL, 4, 128).transpose(2, 1, 0)
            .reshape(128, 4 * BL)).astype(np.float32)
        c0 = np.ascontiguousarray(
            cell[bs].reshape(BL, 4, 128).transpose(2, 1, 0)
            .reshape(128, 4 * BL)).astype(np.float32)
        encl = enc[bs]                                   # [2,S,D]
        encT_re = np.ascontiguousarray(
            encl.reshape(BL, S, 4, 128).transpose(3, 2, 0, 1)
            .reshape(128, 4 * BL * S)).astype(BF16)
        enc_re = np.ascontiguousarray(
            encl.transpose(1, 0, 2).reshape(S, BL * D)).astype(BF16)
        in_maps.append({
            "xg": xg_re, "whh": whh_re, "h0": h0, "c0": c0,
            "encT": encT_re, "enc": enc_re, "whT": whT_re, "wsT": wsT_re,
            "vw1": vw1_re, "vw2": vw2_re, "wsb": wsb_re, "vb": vb_re,
            "vt": vt_re, "vpt": vpt_re, "ones": ones_re,
            "ident": ident_re,
        })
    return in_maps


def kernel(**inputs):
    in_maps = _prep_in_maps(inputs)
    if "nc" not in _cached:
        _cached["nc"] = _build_nc()
    res = bass_utils.run_bass_kernel_spmd(_cached["nc"], in_maps,
                                          core_ids=list(range(NC)))
    outs = [res.results[k]["out"].reshape(BL, T, V) for k in range(NC)]
    full = np.concatenate(outs, axis=0).astype(np.float32)
    full += np.asarray(inputs["Vp_b"]).astype(np.float32)
    return full


if __name__ == "__main__":
    pass



# revision 22
# speedup vs baseline: 1.2458x; 1.1735x over previous
"""AttnOutputDecoder Trainium2 kernel (v2).

Sharding: data-parallel over batch B=16 across 8 cores (2 batches/core).

Structure per core:
- LSTM with W-stationary bf16 matmuls; x@W_ih.T precomputed on host (f32)
  and folded into the gate PSUM via an identity-lhsT f32 matmul (no DVE add).
  Gate order (f,i,o,g) so one sigmoid covers f+i, then tanh(g), then
  sigmoid(o).
- Bahdanau scores via PE broadcast: z[d,(t,s)] = wh[s,d] + ws[t,d] built by
  two selector matmuls into PSUM, then one big amortized tanh (ACT) with
  Ws_b as the per-partition bias, then per-t dot with vt via tiny matmuls
  accumulating e[s,t] in PSUM. All interleaved with LSTM steps.
- Softmax / context / output-projection per 16-step block, pipelined.
- Full-vocab projection streamed in 2000-column blocks (Pool/SWDGE DMAs,
  16KB per partition each); output bf16; Vp_b added on host.
"""

import numpy as np
import ml_dtypes

import concourse.bass as bass
import concourse.mybir as mybir
import concourse.tile as tile
from concourse import bacc
from concourse import bass_utils

BF16 = ml_dtypes.bfloat16
F32 = mybir.dt.float32
BF = mybir.dt.bfloat16
AF = mybir.ActivationFunctionType
ALU = mybir.AluOpType

B, T, S, D, V = 16, 64, 128, 512, 32000
NC = 8
BL = B // NC          # local batches per core = 2
R = BL * T            # local rows = 128
G4 = 4 * D            # 2048 gates
KC = D // 128         # 4 contraction chunks
TB = 16               # t-block for the attention pipeline
NBLK = T // TB
VB = 2000             # vocab block (one DMA; 4 matmul sub-blocks of 500)
NVB = V // VB         # 16 vocab blocks
VSUB = 500            # matmul moving-dim sub-block
NPREB = 5             # vocab blocks prefetched resident in SBUF

# gate row-block permutation: pytorch (i,f,g,o) -> (f,i,o,g), 128-row blocks
GPERM = [4, 5, 6, 7, 0, 1, 2, 3, 12, 13, 14, 15, 8, 9, 10, 11]

_cached = {}
DEBUG_TAPS = False


def _build_nc():
    nc = bacc.Bacc("TRN2", target_bir_lowering=False, debug=False,
                   num_devices=NC)

    def din(name, shape, dt):
        return nc.dram_tensor(name, shape, dt, kind="ExternalInput").ap()

    t_xg = din("xg", [128, 16 * T * BL], F32)        # [p,(j,t,b)] perm order
    t_whh = din("whh", [128, KC * G4], BF)           # [p,(kc,g)] perm order
    t_h0 = din("h0", [128, KC * BL], BF)             # [p,(kc,b)]
    t_c0 = din("c0", [128, KC * BL], F32)
    t_encT = din("encT", [128, KC * BL * S], BF)     # [p,(kc,b,s)]
    t_enc = din("enc", [128, BL * D], BF)            # [s,(b,d)]
    t_whT = din("whT", [128, KC * D], BF)            # [p,(kc,d)] Wh_w.T re
    t_wsT = din("wsT", [128, KC * D], BF)
    t_vw1 = din("vw1", [128, KC * D], BF)            # (V_w[:,:D]).T re
    t_vw2 = din("vw2", [128, KC * D], BF)
    t_wsb = din("wsb", [128, KC], F32)               # Ws_b chunks
    t_vb = din("vb", [128, KC], F32)                 # V_b chunks
    t_vt = din("vt", [128, KC], BF)                  # vt_w chunks
    t_vpt = din("vpt", [128, NVB * KC * VB], BF)     # [p,(nb,kc,v)] Vp_w.T re
    t_selS = din("selS", [128, TB * S], BF)          # delta(s==s') per t
    t_selT = din("selT", [16, TB * S], BF)           # delta(t==t') per s
    t_ident = din("ident", [128, 128], BF)
    t_identF = din("identF", [128, 128], F32)
    t_out = nc.dram_tensor("out", [R, V], BF, kind="ExternalOutput").ap()
    if DEBUG_TAPS:
        t_dbg_h = nc.dram_tensor("dbg_h", [128, KC * BL * T], BF,
                                 kind="ExternalOutput").ap()
        t_dbg_e = nc.dram_tensor("dbg_e", [128, BL * T], F32,
                                 kind="ExternalOutput").ap()
        t_dbg_o2 = nc.dram_tensor("dbg_o2", [128, KC * BL * T], BF,
                                  kind="ExternalOutput").ap()
        t_dbg_ws = nc.dram_tensor("dbg_ws", [128, BL * D], BF,
                                  kind="ExternalOutput").ap()
        t_dbg_wh = nc.dram_tensor("dbg_wh", [128, BL * D], BF,
                                  kind="ExternalOutput").ap()
        t_dbg_th = nc.dram_tensor("dbg_th", [128, 2 * KC * 8 * S], BF,
                                  kind="ExternalOutput").ap()

    with tile.TileContext(nc) as tc:
        with (
            tc.tile_pool(name="const", bufs=1) as cp,
            tc.tile_pool(name="state", bufs=1) as sp,
            tc.tile_pool(name="blk", bufs=2) as bp,
            tc.tile_pool(name="gates", bufs=2) as gp,
            tc.tile_pool(name="th", bufs=6) as thp,
            tc.tile_pool(name="attn", bufs=3) as ap_,
            tc.tile_pool(name="voc", bufs=2) as vp,
        ):
            # ---- resident constants ----
            whh = cp.tile([128, KC * G4], BF)
            nc.sync.dma_start(out=whh[:], in_=t_whh[:])
            xg = cp.tile([128, 16 * T * BL], F32)
            nc.sync.dma_start(out=xg[:], in_=t_xg[:])
            encT = cp.tile([128, KC * BL * S], BF)
            nc.sync.dma_start(out=encT[:], in_=t_encT[:])
            enc = cp.tile([128, BL * D], BF)
            nc.sync.dma_start(out=enc[:], in_=t_enc[:])
            whT = cp.tile([128, KC * D], BF)
            nc.sync.dma_start(out=whT[:], in_=t_whT[:])
            wsT = cp.tile([128, KC * D], BF)
            nc.sync.dma_start(out=wsT[:], in_=t_wsT[:])
            vw1 = cp.tile([128, KC * D], BF)
            nc.sync.dma_start(out=vw1[:], in_=t_vw1[:])
            vw2 = cp.tile([128, KC * D], BF)
            nc.sync.dma_start(out=vw2[:], in_=t_vw2[:])
            wsb = cp.tile([128, KC], F32)
            nc.sync.dma_start(out=wsb[:], in_=t_wsb[:])
            vb = cp.tile([128, KC], F32)
            nc.sync.dma_start(out=vb[:], in_=t_vb[:])
            vt = cp.tile([128, KC], BF)
            nc.sync.dma_start(out=vt[:], in_=t_vt[:])
            selS = cp.tile([128, TB * S], BF)
            nc.sync.dma_start(out=selS[:], in_=t_selS[:])
            selT = cp.tile([16, TB * S], BF)
            nc.sync.dma_start(out=selT[:], in_=t_selT[:])
            ident = cp.tile([128, 128], BF)
            nc.sync.dma_start(out=ident[:], in_=t_ident[:])
            identF = cp.tile([128, 128], F32)
            nc.sync.dma_start(out=identF[:], in_=t_identF[:])

            # ---- state ----
            h0bf = sp.tile([128, KC * BL], BF)
            nc.sync.dma_start(out=h0bf[:], in_=t_h0[:])
            c = sp.tile([128, KC * BL], F32)
            nc.sync.dma_start(out=c[:], in_=t_c0[:])
            outT = sp.tile([128, KC * BL * T], BF)   # [p,(kc,b,t)] all h's
            o2T = sp.tile([128, KC * BL * T], BF)    # [p,(ec,b,t)]
            whsT = sp.tile([128, BL * D], BF)        # [s,(b,d)] wh transposed

            h0bf3 = h0bf[:].rearrange("p (kc b) -> p kc b", kc=KC, b=BL)
            xg4 = xg[:].rearrange("p (j t b) -> p j t b", j=16, t=T, b=BL)
            outT4 = outT[:].rearrange("p (kc b t) -> p kc b t", kc=KC, b=BL,
                                      t=T)

            # ---- vocab weight prefetch (Pool/SWDGE; hidden under compute) --
            vpt4 = t_vpt[:].rearrange("p (nb kc v) -> p nb kc v", nb=NVB,
                                      kc=KC, v=VB)
            vpre = cp.tile([128, NPREB * KC * VB], BF)
            vpre4 = vpre[:].rearrange("p (i kc v) -> p i kc v", i=NPREB,
                                      kc=KC, v=VB)
            for i in range(NPREB):
                nc.gpsimd.dma_start(out=vpre4[:, i], in_=vpt4[:, i])

            with (
                tc.tile_pool(name="ps_g", bufs=1, space="PSUM") as ppg,
                tc.tile_pool(name="ps_z", bufs=2, space="PSUM") as ppz,
                tc.tile_pool(name="ps_e", bufs=1, space="PSUM") as ppe,
                tc.tile_pool(name="ps_sm", bufs=2, space="PSUM") as pps,
            ):
                eps2 = ppe.tile([128, BL * T], F32)  # e[s,(b,t)]
                eps3 = eps2[:].rearrange("p (b t) -> p b t", b=BL, t=T)

                # ---- whsT = (enc @ Wh_w.T) in [s,(b,d)] layout ----
                for b in range(BL):
                    wps = pps.tile([128, 512], F32, tag="sm")
                    for kc in range(KC):
                        nc.tensor.matmul(
                            out=wps[:],
                            lhsT=encT[:, (kc * BL + b) * S:(kc * BL + b + 1) * S],
                            rhs=whT[:, kc * D:(kc + 1) * D],
                            start=(kc == 0), stop=(kc == KC - 1))
                    if b == 0:
                        nc.scalar.copy(out=whsT[:, b * D:(b + 1) * D],
                                       in_=wps[:])
                    else:
                        nc.vector.tensor_copy(out=whsT[:, b * D:(b + 1) * D],
                                              in_=wps[:])

                # ---------- pipelined closures ----------
                def z_chunk(blk, b, half, dc, wstT_sb, th_list):
                    # z[d,(t8,s128)] = wh[s,d]+ws[t,d] ; tanh (Ws_b as bias)
                    zt = ppz.tile([128, 8 * S], F32, tag="zt", name="zt")
                    for g in range(2):
                        so = (half * 8 + g * 4) * S
                        zo = g * 4 * S
                        nc.tensor.matmul(
                            out=zt[:, zo:zo + 4 * S],
                            lhsT=whsT[:, b * D + dc * 128:
                                      b * D + (dc + 1) * 128],
                            rhs=selS[:, so:so + 4 * S],
                            start=True, stop=False)
                        nc.tensor.matmul(
                            out=zt[:, zo:zo + 4 * S],
                            lhsT=wstT_sb[:, b * D + dc * 128:
                                         b * D + (dc + 1) * 128],
                            rhs=selT[:, so:so + 4 * S],
                            start=False, stop=True)
                    th = thp.tile([128, 8 * S], BF, tag="th", name="th")
                    nc.scalar.activation(out=th[:], in_=zt[:], func=AF.Tanh,
                                         bias=wsb[:, dc:dc + 1])
                    if DEBUG_TAPS and blk == 0 and b == 0:
                        off = (half * KC + dc) * 8 * S
                        nc.sync.dma_start(
                            out=t_dbg_th[:, off:off + 8 * S], in_=th[:])
                    th_list[dc] = th

                def dots(blk, b, half, th_list):
                    # e[s,t] = sum_d th[d,(t,s)]*vt[d]; one complete
                    # 4-chunk PSUM group per t (PSUM zero-regions forbid
                    # interleaved groups within a bank)
                    tlo = blk * TB
                    for tl in range(8):
                        t = tlo + half * 8 + tl
                        for dc in range(KC):
                            nc.tensor.matmul(
                                out=eps3[:, b, t:t + 1],
                                lhsT=th_list[dc][:, tl * S:(tl + 1) * S],
                                rhs=vt[:, dc:dc + 1],
                                start=(dc == 0), stop=(dc == KC - 1))

                def softmax(blk, b, atb):
                    tlo = blk * TB
                    ebf = ap_.tile([128, TB], BF, tag="ebf", name="ebf")
                    nc.scalar.activation(out=ebf[:],
                                         in_=eps3[:, b, tlo:tlo + TB],
                                         func=AF.Exp)
                    etp = pps.tile([TB, S], BF, tag="sm", name="etp")
                    nc.tensor.transpose(out=etp[:], in_=ebf[:],
                                        identity=ident[:, :])
                    ssum = ap_.tile([TB, 1], F32, tag="ssum", name="ssum")
                    nc.vector.tensor_reduce(out=ssum[:], in_=etp[:],
                                            axis=mybir.AxisListType.X,
                                            op=ALU.add)
                    rsum = ap_.tile([TB, 1], F32, tag="rsum", name="rsum")
                    nc.vector.reciprocal(out=rsum[:], in_=ssum[:])
                    abf = ap_.tile([TB, S], BF, tag="abf", name="abf")
                    nc.vector.tensor_scalar_mul(out=abf[:], in0=etp[:],
                                                scalar1=rsum[:])
                    atp = pps.tile([S, TB], BF, tag="sm", name="atp")
                    nc.tensor.transpose(out=atp[:], in_=abf[:],
                                        identity=ident[0:TB, 0:TB])
                    nc.vector.tensor_copy(out=atb[:, b * TB:(b + 1) * TB],
                                          in_=atp[:])

                def ctx(blk, b, dc, atb, ctxT):
                    cps = pps.tile([128, TB], F32, tag="sm", name="cps")
                    nc.tensor.matmul(out=cps[:],
                                     lhsT=enc[:, b * D + dc * 128:
                                              b * D + (dc + 1) * 128],
                                     rhs=atb[:, b * TB:(b + 1) * TB],
                                     start=True, stop=True)
                    if (dc + b) % 2 == 0:
                        nc.scalar.copy(
                            out=ctxT[:, (b * KC + dc) * TB:
                                     (b * KC + dc + 1) * TB], in_=cps[:])
                    else:
                        nc.vector.tensor_copy(
                            out=ctxT[:, (b * KC + dc) * TB:
                                     (b * KC + dc + 1) * TB], in_=cps[:])

                def o2(blk, b, ec, ctxT):
                    tlo = blk * TB
                    ops = pps.tile([128, TB], F32, tag="sm", name="ops")
                    for kc in range(KC):
                        nc.tensor.matmul(
                            out=ops[:],
                            lhsT=vw1[:, kc * D + ec * 128:
                                     kc * D + (ec + 1) * 128],
                            rhs=ctxT[:, (b * KC + kc) * TB:
                                     (b * KC + kc + 1) * TB],
                            start=(kc == 0), stop=False)
                    for kc in range(KC):
                        nc.tensor.matmul(
                            out=ops[:],
                            lhsT=vw2[:, kc * D + ec * 128:
                                     kc * D + (ec + 1) * 128],
                            rhs=outT[:, (kc * BL + b) * T + tlo:
                                     (kc * BL + b) * T + tlo + TB],
                            start=False, stop=(kc == KC - 1))
                    nc.vector.tensor_scalar(
                        out=o2T[:, ec * R + b * T + tlo:
                                ec * R + b * T + tlo + TB],
                        in0=ops[:], scalar1=vb[:, ec:ec + 1], scalar2=None,
                        op0=ALU.add)

                pending = []
                last_wst = [None]

                def drain(n):
                    for _ in range(min(n, len(pending))):
                        pending.pop(0)()

                # ================== LSTM + pipeline ==================
                for blk in range(NBLK):
                    tlo = blk * TB
                    for t in range(tlo, tlo + TB):
                        gps = ppg.tile([128, 32], F32, tag="gps", name="gps")
                        for j in range(16):
                            for kc in range(KC):
                                rhs = (h0bf3[:, kc, :] if t == 0
                                       else outT4[:, kc, :, t - 1])
                                nc.tensor.matmul(
                                    out=gps[:, 2 * j:2 * j + 2],
                                    lhsT=whh[:, kc * G4 + j * 128:
                                              kc * G4 + (j + 1) * 128],
                                    rhs=rhs, start=(kc == 0), stop=False)
                            nc.tensor.matmul(out=gps[:, 2 * j:2 * j + 2],
                                             lhsT=identF[:],
                                             rhs=xg4[:, j, t, :],
                                             start=False, stop=True)
                        sio = gp.tile([128, 24], F32, tag="sio", name="sio")
                        nc.scalar.activation(out=sio[:, 0:16],
                                             in_=gps[:, 0:16],
                                             func=AF.Sigmoid)
                        tg = gp.tile([128, 8], F32, tag="tg", name="tg")
                        nc.scalar.activation(out=tg[:], in_=gps[:, 24:32],
                                             func=AF.Tanh)
                        nc.scalar.activation(out=sio[:, 16:24],
                                             in_=gps[:, 16:24],
                                             func=AF.Sigmoid)
                        t1 = gp.tile([128, 8], F32, tag="t1", name="t1")
                        t2 = gp.tile([128, 8], F32, tag="t2", name="t2")
                        nc.vector.tensor_mul(out=t1[:], in0=sio[:, 0:8],
                                             in1=c[:])
                        nc.vector.tensor_mul(out=t2[:], in0=sio[:, 8:16],
                                             in1=tg[:])
                        nc.vector.tensor_add(out=c[:], in0=t1[:], in1=t2[:])
                        tc_ = gp.tile([128, 8], F32, tag="tc", name="tc")
                        nc.scalar.activation(out=tc_[:], in_=c[:],
                                             func=AF.Tanh)
                        sio3 = sio[:, 16:24].rearrange(
                            "p (kc b) -> p kc b", kc=KC, b=BL)
                        tc3 = tc_[:].rearrange("p (kc b) -> p kc b",
                                               kc=KC, b=BL)
                        nc.vector.tensor_mul(out=outT4[:, :, :, t],
                                             in0=sio3, in1=tc3)
                        drain(3)

                    # ws for this t-block, transposed: wstT[t16,(b,d)]
                    wstT_sb = bp.tile([16, BL * D], BF, tag="wstT",
                                      name="wstT")
                    last_wst[0] = wstT_sb
                    for b in range(BL):
                        wps = pps.tile([16, 512], F32, tag="sm", name="wps")
                        for kc in range(KC):
                            nc.tensor.matmul(
                                out=wps[:],
                                lhsT=outT[:, (kc * BL + b) * T + tlo:
                                          (kc * BL + b) * T + tlo + TB],
                                rhs=wsT[:, kc * D:(kc + 1) * D],
                                start=(kc == 0), stop=(kc == KC - 1))
                        if b == 0:
                            nc.scalar.copy(out=wstT_sb[:, b * D:(b + 1) * D],
                                           in_=wps[:])
                        else:
                            nc.vector.tensor_copy(
                                out=wstT_sb[:, b * D:(b + 1) * D], in_=wps[:])

                    atb = bp.tile([128, BL * TB], BF, tag="atb", name="atb")
                    ctxT = bp.tile([128, BL * KC * TB], BF, tag="ctxT",
                                   name="ctxT")
                    for b in range(BL):
                        for half in range(2):
                            th_list = [None] * KC
                            for dc in range(KC):
                                pending.append(
                                    (lambda blk=blk, b=b, half=half, dc=dc,
                                     w=wstT_sb, tl=th_list:
                                     z_chunk(blk, b, half, dc, w, tl)))
                            pending.append(
                                (lambda blk=blk, b=b, half=half, tl=th_list:
                                 dots(blk, b, half, tl)))
                        pending.append(
                            lambda blk=blk, b=b, a=atb: softmax(blk, b, a))
                        for dc in range(KC):
                            pending.append(
                                (lambda blk=blk, b=b, dc=dc, a=atb, cx=ctxT:
                                 ctx(blk, b, dc, a, cx)))
                        for ec in range(KC):
                            pending.append(
                                (lambda blk=blk, b=b, ec=ec, cx=ctxT:
                                 o2(blk, b, ec, cx)))

                drain(len(pending))

                if DEBUG_TAPS:
                    nc.sync.dma_start(out=t_dbg_h[:], in_=outT[:])
                    nc.sync.dma_start(out=t_dbg_o2[:], in_=o2T[:])
                    nc.sync.dma_start(out=t_dbg_wh[:], in_=whsT[:])
                    nc.sync.dma_start(out=t_dbg_ws[0:16, :],
                                      in_=last_wst[0][:])
                    epssb = sp.tile([128, BL * T], F32)
                    nc.vector.tensor_copy(out=epssb[:], in_=eps2[:])
                    nc.sync.dma_start(out=t_dbg_e[:], in_=epssb[:])

            # ================= vocab projection =================
            # Vp_b is added on the host; output is bf16 (upcast on host).
            with tc.tile_pool(name="ps_v", bufs=2, space="PSUM") as ppv:
                for ib in range(NVB):
                    v0 = ib * VB
                    if ib < NPREB:
                        vsrc = vpre4[:, ib]
                    else:
                        vps = vp.tile([128, KC, VB], BF, tag="vps",
                                      name="vps")
                        nc.gpsimd.dma_start(out=vps[:], in_=vpt4[:, ib])
                        vsrc = vps
                    lsb = vp.tile([128, VB], BF, tag="lsb", bufs=3,
                                  name="lsb")
                    for sb in range(VB // VSUB):
                        s0 = sb * VSUB
                        lps = ppv.tile([128, VSUB], F32, tag="lps",
                                       name="lps")
                        for kc in range(KC):
                            nc.tensor.matmul(
                                out=lps[:],
                                lhsT=o2T[:, kc * 128:(kc + 1) * 128],
                                rhs=vsrc[:, kc, s0:s0 + VSUB],
                                start=(kc == 0), stop=(kc == KC - 1))
                        if sb % 2 == 0:
                            nc.scalar.copy(out=lsb[:, s0:s0 + VSUB],
                                           in_=lps[:])
                        else:
                            nc.vector.tensor_copy(out=lsb[:, s0:s0 + VSUB],
                                                  in_=lps[:])
                    nc.gpsimd.dma_start(out=t_out[:, v0:v0 + VB], in_=lsb[:])

    nc.compile()
    return nc


def _prep_in_maps(inputs):
    inp = {k: np.asarray(v) for k, v in inputs.items()}
    words = inp["words"].astype(np.int64)
    enc = inp["encoder_output"].astype(np.float32)
    pre_h, cell = inp["pre_h"], inp["cell"]
    emb = inp["emb"]
    W_ih, W_hh = inp["W_ih"], inp["W_hh"]
    b_ih, b_hh = inp["b_ih"], inp["b_hh"]
    Wh_w = inp["Wh_w"]
    Ws_w, Ws_b = inp["Ws_w"], inp["Ws_b"]
    vt_w = inp["vt_w"]
    V_w, V_b = inp["V_w"], inp["V_b"]
    Vp_w = inp["Vp_w"]

    P = np.array(GPERM)
    W_hh2 = W_hh.reshape(16, 128, D)[P].reshape(G4, D)
    W_ih2 = W_ih.reshape(16, 128, D)[P].reshape(G4, D)
    bias2 = (b_ih + b_hh).astype(np.float32).reshape(16, 128)[P].reshape(G4)

    def re_lhsT(m):  # [512, N] -> [128, 4*N] chunk-major, bf16
        n = m.shape[1]
        return np.ascontiguousarray(
            m.reshape(4, 128, n).transpose(1, 0, 2).reshape(128, 4 * n)
        ).astype(BF16)

    whh_re = re_lhsT(np.ascontiguousarray(W_hh2.T))
    whT_re = re_lhsT(np.ascontiguousarray(Wh_w.T))
    wsT_re = re_lhsT(np.ascontiguousarray(Ws_w.T))
    vw1_re = re_lhsT(np.ascontiguousarray(V_w[:, :D].T))
    vw2_re = re_lhsT(np.ascontiguousarray(V_w[:, D:].T))
    # [128,(kc,v)] -> [128,(nb,kc,vb)] so each vocab block is one contiguous
    # 16KB-per-partition DMA
    vpt_re = np.ascontiguousarray(
        re_lhsT(np.ascontiguousarray(Vp_w.T))
        .reshape(128, KC, NVB, VB).transpose(0, 2, 1, 3)
        .reshape(128, NVB * KC * VB))
    wsb_re = np.ascontiguousarray(Ws_b.reshape(4, 128).T).astype(np.float32)
    vb_re = np.ascontiguousarray(V_b.reshape(4, 128).T).astype(np.float32)
    vt_re = np.ascontiguousarray(vt_w.reshape(4, 128).T).astype(BF16)
    # selS[s', (t,s)] = (s == s')
    selS_re = np.zeros((S, TB * S), dtype=BF16)
    for tt in range(TB):
        selS_re[:, tt * S:(tt + 1) * S] = np.eye(S, dtype=BF16)
    # selT[t', (t,s)] = (t == t')
    selT_re = np.zeros((16, TB * S), dtype=BF16)
    for tt in range(TB):
        selT_re[tt, tt * S:(tt + 1) * S] = 1.0
    ident_re = np.eye(128, dtype=np.float32).astype(BF16)
    identF_re = np.eye(128, dtype=np.float32)

    x_all = emb[words]                                    # [B,T,D]
    xg_all = x_all @ W_ih2.T.astype(np.float32) + bias2   # [B,T,4D]

    in_maps = []
    for k in range(NC):
        bs = slice(k * BL, (k + 1) * BL)
        xgl = xg_all[bs]                                 # [2,T,2048]
        xg_re = np.ascontiguousarray(
            xgl.reshape(BL, T, 16, 128).transpose(3, 2, 1, 0)
            .reshape(128, 16 * T * BL)).astype(np.float32)
        h0 = np.ascontiguousarray(
            pre_h[bs].reshape(BL, 4, 128).transpose(2, 1, 0)
            .reshape(128, 4 * BL)).astype(BF16)
        c0 = np.ascontiguousarray(
            cell[bs].reshape(BL, 4, 128).transpose(2, 1, 0)
            .reshape(128, 4 * BL)).astype(np.float32)
        encl = enc[bs]                                   # [2,S,D]
        encT_re = np.ascontiguousarray(
            encl.reshape(BL, S, 4, 128).transpose(3, 2, 0, 1)
            .reshape(128, 4 * BL * S)).astype(BF16)
        enc_re = np.ascontiguousarray(
            encl.transpose(1, 0, 2).reshape(S, BL * D)).astype(BF16)
        in_maps.append({
            "xg": xg_re, "whh": whh_re, "h0": h0, "c0": c0,
            "encT": encT_re, "enc": enc_re, "whT": whT_re, "wsT": wsT_re,
            "vw1": vw1_re, "vw2": vw2_re, "wsb": wsb_re, "vb": vb_re,
            "vt": vt_re, "vpt": vpt_re, "selS": selS_re, "selT": selT_re,
            "ident": ident_re, "identF": identF_re,
        })
    return in_maps


def kernel(**inputs):
    in_maps = _prep_in_maps(inputs)
    if "nc" not in _cached:
        _cached["nc"] = _build_nc()
    res = bass_utils.run_bass_kernel_spmd(_cached["nc"], in_maps,
                                          core_ids=list(range(NC)))
    outs = [res.results[k]["out"].reshape(BL, T, V) for k in range(NC)]
    full = np.concatenate(outs, axis=0).astype(np.float32)
    full += np.asarray(inputs["Vp_b"]).astype(np.float32)
    return full


if __name__ == "__main__":
    pass


# revision 30
# speedup vs baseline: 1.4152x; 1.1360x over previous
"""AttnOutputDecoder Trainium2 kernel (v2).

Sharding: data-parallel over batch B=16 across 8 cores (2 batches/core).

Structure per core:
- LSTM with W-stationary bf16 matmuls; x@W_ih.T precomputed on host (f32)
  and folded into the gate PSUM via an identity-lhsT f32 matmul (no DVE add).
  Gate order (f,i,o,g) so one sigmoid covers f+i, then tanh(g), then
  sigmoid(o).
- Bahdanau scores via PE broadcast: z[d,(t,s)] = wh[s,d] + ws[t,d] built by
  two selector matmuls into PSUM, then one big amortized tanh (ACT) with
  Ws_b as the per-partition bias, then per-t dot with vt via tiny matmuls
  accumulating e[s,t] in PSUM. All interleaved with LSTM steps.
- Softmax / context / output-projection per 16-step block, pipelined.
- Full-vocab projection streamed in 2000-column blocks (Pool/SWDGE DMAs,
  16KB per partition each); output bf16; Vp_b added on host.
"""

import numpy as np
import ml_dtypes

import concourse.bass as bass
import concourse.mybir as mybir
import concourse.tile as tile
from concourse import bacc
from concourse import bass_utils

BF16 = ml_dtypes.bfloat16
F32 = mybir.dt.float32
BF = mybir.dt.bfloat16
AF = mybir.ActivationFunctionType
ALU = mybir.AluOpType

B, T, S, D, V = 16, 64, 128, 512, 32000
NC = 8
BL = B // NC          # local batches per core = 2
R = BL * T            # local rows = 128
G4 = 4 * D            # 2048 gates
KC = D // 128         # 4 contraction chunks
TB = 16               # t-block for the attention pipeline
NBLK = T // TB
VB = 2000             # vocab block (one DMA; 4 matmul sub-blocks of 500)
NVB = V // VB         # 16 vocab blocks
VSUB = 500            # matmul moving-dim sub-block
NPREB = 5             # vocab blocks prefetched resident in SBUF

# gate row-block permutation: pytorch (i,f,g,o) -> (f,i,o,g), 128-row blocks
GPERM = [4, 5, 6, 7, 0, 1, 2, 3, 12, 13, 14, 15, 8, 9, 10, 11]

_cached = {}
DEBUG_TAPS = False


def _build_nc():
    nc = bacc.Bacc("TRN2", target_bir_lowering=False, debug=False,
                   num_devices=NC)

    def din(name, shape, dt):
        return nc.dram_tensor(name, shape, dt, kind="ExternalInput").ap()

    t_xg = din("xg", [128, 16 * T * BL], F32)        # [p,(j,t,b)] perm order
    t_whh = din("whh", [128, KC * G4], BF)           # [p,(kc,g)] perm order
    t_h0 = din("h0", [128, KC * BL], BF)             # [p,(kc,b)]
    t_c0 = din("c0", [128, KC * BL], F32)
    t_encT = din("encT", [128, KC * BL * S], BF)     # [p,(kc,b,s)]
    t_enc = din("enc", [128, BL * D], BF)            # [s,(b,d)]
    t_whT = din("whT", [128, KC * D], BF)            # [p,(kc,d)] Wh_w.T re
    t_wsT = din("wsT", [128, KC * D], BF)
    t_vw1 = din("vw1", [128, KC * D], BF)            # (V_w[:,:D]).T re
    t_vw2 = din("vw2", [128, KC * D], BF)
    t_wsb = din("wsb", [128, KC], F32)               # Ws_b chunks
    t_vb = din("vb", [128, KC], F32)                 # V_b chunks
    t_vt = din("vt", [128, KC], BF)                  # vt_w chunks
    t_vpt = din("vpt", [128, NVB * KC * VB], BF)     # [p,(nb,kc,v)] Vp_w.T re
    t_selS = din("selS", [128, TB * S], BF)          # delta(s==s') per t
    t_selT = din("selT", [16, TB * S], BF)           # delta(t==t') per s
    t_ident = din("ident", [128, 128], BF)
    t_identF = din("identF", [128, 128], F32)
    t_out = nc.dram_tensor("out", [R, V], BF, kind="ExternalOutput").ap()
    if DEBUG_TAPS:
        t_dbg_h = nc.dram_tensor("dbg_h", [128, KC * BL * T], BF,
                                 kind="ExternalOutput").ap()
        t_dbg_e = nc.dram_tensor("dbg_e", [128, BL * T], F32,
                                 kind="ExternalOutput").ap()
        t_dbg_o2 = nc.dram_tensor("dbg_o2", [128, KC * BL * T], BF,
                                  kind="ExternalOutput").ap()
        t_dbg_ws = nc.dram_tensor("dbg_ws", [128, BL * D], BF,
                                  kind="ExternalOutput").ap()
        t_dbg_wh = nc.dram_tensor("dbg_wh", [128, BL * D], BF,
                                  kind="ExternalOutput").ap()
        t_dbg_th = nc.dram_tensor("dbg_th", [128, 2 * KC * 8 * S], BF,
                                  kind="ExternalOutput").ap()

    with tile.TileContext(nc) as tc:
        with (
            tc.tile_pool(name="const", bufs=1) as cp,
            tc.tile_pool(name="state", bufs=1) as sp,
            tc.tile_pool(name="blk", bufs=2) as bp,
            tc.tile_pool(name="gates", bufs=2) as gp,
            tc.tile_pool(name="th", bufs=6) as thp,
            tc.tile_pool(name="attn", bufs=3) as ap_,
            tc.tile_pool(name="voc", bufs=2) as vp,
        ):
            # ---- resident constants (DMA order = LSTM-critical first) ----
            whh = cp.tile([128, KC * G4], BF)
            nc.sync.dma_start(out=whh[:], in_=t_whh[:])
            xg = cp.tile([128, 16 * T * BL], F32)
            xgD = t_xg[:].rearrange("p (t j b) -> p t j b", j=16, t=T, b=BL)
            xg4 = xg[:].rearrange("p (t j b) -> p t j b", j=16, t=T, b=BL)
            nc.sync.dma_start(out=xg4[:, 0:TB], in_=xgD[:, 0:TB])
            h0bf = sp.tile([128, KC * BL], BF)
            nc.sync.dma_start(out=h0bf[:], in_=t_h0[:])
            c = sp.tile([128, KC * BL], F32)
            nc.sync.dma_start(out=c[:], in_=t_c0[:])
            identF = cp.tile([128, 128], F32)
            nc.sync.dma_start(out=identF[:], in_=t_identF[:])
            wsb = cp.tile([128, KC], F32)
            nc.sync.dma_start(out=wsb[:], in_=t_wsb[:])
            vb = cp.tile([128, KC], F32)
            nc.sync.dma_start(out=vb[:], in_=t_vb[:])
            vt = cp.tile([128, KC], BF)
            nc.sync.dma_start(out=vt[:], in_=t_vt[:])
            for tchunk in range(1, 4):
                nc.sync.dma_start(out=xg4[:, tchunk * TB:(tchunk + 1) * TB],
                                  in_=xgD[:, tchunk * TB:(tchunk + 1) * TB])
            encT = cp.tile([128, KC * BL * S], BF)
            nc.sync.dma_start(out=encT[:], in_=t_encT[:])
            whT = cp.tile([128, KC * D], BF)
            nc.sync.dma_start(out=whT[:], in_=t_whT[:])
            wsT = cp.tile([128, KC * D], BF)
            nc.sync.dma_start(out=wsT[:], in_=t_wsT[:])
            selS = cp.tile([128, TB * S], BF)
            nc.sync.dma_start(out=selS[:], in_=t_selS[:])
            selT = cp.tile([16, TB * S], BF)
            nc.sync.dma_start(out=selT[:], in_=t_selT[:])
            enc = cp.tile([128, BL * D], BF)
            nc.sync.dma_start(out=enc[:], in_=t_enc[:])
            vw1 = cp.tile([128, KC * D], BF)
            nc.sync.dma_start(out=vw1[:], in_=t_vw1[:])
            vw2 = cp.tile([128, KC * D], BF)
            nc.sync.dma_start(out=vw2[:], in_=t_vw2[:])
            ident = cp.tile([128, 128], BF)
            nc.sync.dma_start(out=ident[:], in_=t_ident[:])

            # ---- state ----
            outT = sp.tile([128, KC * BL * T], BF)   # [p,(kc,b,t)] all h's
            o2T = sp.tile([128, KC * BL * T], BF)    # [p,(ec,b,t)]
            whsT = sp.tile([128, BL * D], BF)        # [s,(b,d)] wh transposed

            h0bf3 = h0bf[:].rearrange("p (kc b) -> p kc b", kc=KC, b=BL)
            outT4 = outT[:].rearrange("p (kc b t) -> p kc b t", kc=KC, b=BL,
                                      t=T)

            # ---- vocab weight prefetch (Pool/SWDGE; hidden under compute) --
            vpt4 = t_vpt[:].rearrange("p (nb kc v) -> p nb kc v", nb=NVB,
                                      kc=KC, v=VB)
            vpre = cp.tile([128, NPREB * KC * VB], BF)
            vpre4 = vpre[:].rearrange("p (i kc v) -> p i kc v", i=NPREB,
                                      kc=KC, v=VB)
            for i in range(NPREB):
                nc.gpsimd.dma_start(out=vpre4[:, i], in_=vpt4[:, i])

            with (
                tc.tile_pool(name="ps_g", bufs=1, space="PSUM") as ppg,
                tc.tile_pool(name="ps_z", bufs=2, space="PSUM") as ppz,
                tc.tile_pool(name="ps_e", bufs=1, space="PSUM") as ppe,
                tc.tile_pool(name="ps_sm", bufs=2, space="PSUM") as pps,
            ):
                eps2 = ppe.tile([128, BL * T], F32)  # e[s,(b,t)]
                eps3 = eps2[:].rearrange("p (b t) -> p b t", b=BL, t=T)

                # ---------- pipelined closures (engine-split queues) -------
                pend_pe = []
                pend_act = []

                def whsT_calc(b):
                    # whsT = (enc @ Wh_w.T) in [s,(b,d)] layout
                    wps = pps.tile([128, 512], F32, tag="sm", name="wps")
                    for kc in range(KC):
                        nc.tensor.matmul(
                            out=wps[:],
                            lhsT=encT[:, (kc * BL + b) * S:(kc * BL + b + 1) * S],
                            rhs=whT[:, kc * D:(kc + 1) * D],
                            start=(kc == 0), stop=(kc == KC - 1))
                    nc.vector.tensor_copy(out=whsT[:, b * D:(b + 1) * D],
                                          in_=wps[:])

                def z_chunk(blk, b, half, dc, wstT_sb, th_list, cnt):
                    # z[d,(t8,s128)] = wh[s,d]+ws[t,d] (PE broadcast matmuls)
                    zt = ppz.tile([128, 8 * S], F32, tag="zt", name="zt")
                    for g in range(2):
                        so = (half * 8 + g * 4) * S
                        zo = g * 4 * S
                        nc.tensor.matmul(
                            out=zt[:, zo:zo + 4 * S],
                            lhsT=whsT[:, b * D + dc * 128:
                                      b * D + (dc + 1) * 128],
                            rhs=selS[:, so:so + 4 * S],
                            start=True, stop=False)
                        nc.tensor.matmul(
                            out=zt[:, zo:zo + 4 * S],
                            lhsT=wstT_sb[:, b * D + dc * 128:
                                         b * D + (dc + 1) * 128],
                            rhs=selT[:, so:so + 4 * S],
                            start=False, stop=True)
                    pend_act.append(
                        lambda: z_tanh(blk, b, half, dc, zt, th_list, cnt))

                def z_tanh(blk, b, half, dc, zt, th_list, cnt):
                    th = thp.tile([128, 8 * S], BF, tag="th", name="th")
                    nc.scalar.activation(out=th[:], in_=zt[:], func=AF.Tanh,
                                         bias=wsb[:, dc:dc + 1])
                    if DEBUG_TAPS and blk == 0 and b == 0:
                        off = (half * KC + dc) * 8 * S
                        nc.sync.dma_start(
                            out=t_dbg_th[:, off:off + 8 * S], in_=th[:])
                    th_list[dc] = th
                    cnt[0] += 1
                    if cnt[0] == KC:
                        pend_pe.append(lambda: dots(blk, b, half, th_list))

                def dots(blk, b, half, th_list):
                    # e[s,t] = sum_d th[d,(t,s)]*vt[d]; one complete
                    # 4-chunk PSUM group per t (PSUM zero-regions forbid
                    # interleaved groups within a bank)
                    tlo = blk * TB
                    for tl in range(8):
                        t = tlo + half * 8 + tl
                        for dc in range(KC):
                            nc.tensor.matmul(
                                out=eps3[:, b, t:t + 1],
                                lhsT=th_list[dc][:, tl * S:(tl + 1) * S],
                                rhs=vt[:, dc:dc + 1],
                                start=(dc == 0), stop=(dc == KC - 1))
                    if half == 1:
                        ab, cx = blkstate[(blk, b)]
                        pend_act.append(lambda: softmax(blk, b, ab, cx))

                def softmax(blk, b, atb, ctxT):
                    # exp(x) = (1+u)/(1-u), u = tanh(x/2): stays in the
                    # sigmoid/tanh ACT table (no act-table reload)
                    tlo = blk * TB
                    ebf = ap_.tile([128, TB], BF, tag="ebf", name="ebf")
                    nc.scalar.activation(out=ebf[:],
                                         in_=eps3[:, b, tlo:tlo + TB],
                                         func=AF.Tanh, scale=0.5)
                    etp = pps.tile([TB, S], BF, tag="sm", name="etp")
                    nc.tensor.transpose(out=etp[:], in_=ebf[:],
                                        identity=ident[:, :])
                    # num = 1+u ; den = 1-u ; e^x = num/den
                    num = ap_.tile([TB, S], F32, tag="num", name="num")
                    nc.vector.tensor_scalar(out=num[:], in0=etp[:],
                                            scalar1=1.0, scalar2=None,
                                            op0=ALU.add)
                    den = ap_.tile([TB, S], F32, tag="den", name="den")
                    nc.vector.tensor_scalar(out=den[:], in0=etp[:],
                                            scalar1=-1.0, scalar2=1.0,
                                            op0=ALU.mult, op1=ALU.add)
                    rden = ap_.tile([TB, S], F32, tag="rden", name="rden")
                    nc.vector.reciprocal(out=rden[:], in_=den[:])
                    ex = ap_.tile([TB, S], F32, tag="ex", name="ex")
                    nc.vector.tensor_mul(out=ex[:], in0=num[:], in1=rden[:])
                    ssum = ap_.tile([TB, 1], F32, tag="ssum", name="ssum")
                    nc.vector.tensor_reduce(out=ssum[:], in_=ex[:],
                                            axis=mybir.AxisListType.X,
                                            op=ALU.add)
                    rsum = ap_.tile([TB, 1], F32, tag="rsum", name="rsum")
                    nc.vector.reciprocal(out=rsum[:], in_=ssum[:])
                    abf = ap_.tile([TB, S], BF, tag="abf", name="abf")
                    nc.vector.tensor_scalar_mul(out=abf[:], in0=ex[:],
                                                scalar1=rsum[:])
                    atp = pps.tile([S, TB], BF, tag="sm", name="atp")
                    nc.tensor.transpose(out=atp[:], in_=abf[:],
                                        identity=ident[0:TB, 0:TB])
                    nc.vector.tensor_copy(out=atb[:, b * TB:(b + 1) * TB],
                                          in_=atp[:])
                    for dc in range(KC):
                        pend_pe.append(
                            (lambda dc=dc: ctx(blk, b, dc, atb, ctxT)))

                def ctx(blk, b, dc, atb, ctxT):
                    cps = pps.tile([128, TB], F32, tag="sm", name="cps")
                    nc.tensor.matmul(out=cps[:],
                                     lhsT=enc[:, b * D + dc * 128:
                                              b * D + (dc + 1) * 128],
                                     rhs=atb[:, b * TB:(b + 1) * TB],
                                     start=True, stop=True)
                    if (dc + b) % 2 == 0:
                        nc.scalar.copy(
                            out=ctxT[:, (b * KC + dc) * TB:
                                     (b * KC + dc + 1) * TB], in_=cps[:])
                    else:
                        nc.vector.tensor_copy(
                            out=ctxT[:, (b * KC + dc) * TB:
                                     (b * KC + dc + 1) * TB], in_=cps[:])
                    if dc == KC - 1:
                        for ec in range(KC):
                            pend_pe.append(
                                (lambda ec=ec: o2(blk, b, ec, ctxT)))

                def o2(blk, b, ec, ctxT):
                    tlo = blk * TB
                    ops = pps.tile([128, TB], F32, tag="sm", name="ops")
                    for kc in range(KC):
                        nc.tensor.matmul(
                            out=ops[:],
                            lhsT=vw1[:, kc * D + ec * 128:
                                     kc * D + (ec + 1) * 128],
                            rhs=ctxT[:, (b * KC + kc) * TB:
                                     (b * KC + kc + 1) * TB],
                            start=(kc == 0), stop=False)
                    for kc in range(KC):
                        nc.tensor.matmul(
                            out=ops[:],
                            lhsT=vw2[:, kc * D + ec * 128:
                                     kc * D + (ec + 1) * 128],
                            rhs=outT[:, (kc * BL + b) * T + tlo:
                                     (kc * BL + b) * T + tlo + TB],
                            start=False, stop=(kc == KC - 1))
                    nc.vector.tensor_scalar(
                        out=o2T[:, ec * R + b * T + tlo:
                                ec * R + b * T + tlo + TB],
                        in0=ops[:], scalar1=vb[:, ec:ec + 1], scalar2=None,
                        op0=ALU.add)

                blkstate = {}
                last_wst = [None]

                def drain_pe(n):
                    for _ in range(min(n, len(pend_pe))):
                        pend_pe.pop(0)()

                def drain_act(n):
                    for _ in range(min(n, len(pend_act))):
                        pend_act.pop(0)()

                # ================== LSTM + pipeline ==================
                for blk in range(NBLK):
                    tlo = blk * TB
                    for t in range(tlo, tlo + TB):
                        gps = ppg.tile([128, 32], F32, tag="gps", name="gps")
                        for j in range(16):
                            for kc in range(KC):
                                rhs = (h0bf3[:, kc, :] if t == 0
                                       else outT4[:, kc, :, t - 1])
                                nc.tensor.matmul(
                                    out=gps[:, 2 * j:2 * j + 2],
                                    lhsT=whh[:, kc * G4 + j * 128:
                                              kc * G4 + (j + 1) * 128],
                                    rhs=rhs, start=(kc == 0), stop=False)
                            nc.tensor.matmul(out=gps[:, 2 * j:2 * j + 2],
                                             lhsT=identF[:],
                                             rhs=xg4[:, t, j, :],
                                             start=False, stop=True)
                        # PE-side pipeline work queues behind the (chain-
                        # critical) gate matmuls, ahead of next step's gates
                        drain_pe(3)
                        sio = gp.tile([128, 24], F32, tag="sio", name="sio")
                        nc.scalar.activation(out=sio[:, 0:16],
                                             in_=gps[:, 0:16],
                                             func=AF.Sigmoid)
                        tg = gp.tile([128, 8], F32, tag="tg", name="tg")
                        nc.scalar.activation(out=tg[:], in_=gps[:, 24:32],
                                             func=AF.Tanh)
                        nc.scalar.activation(out=sio[:, 16:24],
                                             in_=gps[:, 16:24],
                                             func=AF.Sigmoid)
                        t1 = gp.tile([128, 8], F32, tag="t1", name="t1")
                        t2 = gp.tile([128, 8], F32, tag="t2", name="t2")
                        nc.vector.tensor_mul(out=t1[:], in0=sio[:, 0:8],
                                             in1=c[:])
                        nc.vector.tensor_mul(out=t2[:], in0=sio[:, 8:16],
                                             in1=tg[:])
                        nc.vector.tensor_add(out=c[:], in0=t1[:], in1=t2[:])
                        tc_ = gp.tile([128, 8], F32, tag="tc", name="tc")
                        nc.scalar.activation(out=tc_[:], in_=c[:],
                                             func=AF.Tanh)
                        sio3 = sio[:, 16:24].rearrange(
                            "p (kc b) -> p kc b", kc=KC, b=BL)
                        tc3 = tc_[:].rearrange("p (kc b) -> p kc b",
                                               kc=KC, b=BL)
                        nc.vector.tensor_mul(out=outT4[:, :, :, t],
                                             in0=sio3, in1=tc3)
                        # ACT-side pipeline work after the step's ACT chain
                        drain_act(2 if t % 2 == 0 else 1)

                    if blk == 0:
                        # deferred so startup DMAs don't gate LSTM step 0
                        for b in range(BL):
                            pend_pe.append(lambda b=b: whsT_calc(b))

                    # ws for this t-block, transposed: wstT[t16,(b,d)]
                    wstT_sb = bp.tile([16, BL * D], BF, tag="wstT",
                                      name="wstT")
                    last_wst[0] = wstT_sb
                    for b in range(BL):
                        wps = pps.tile([16, 512], F32, tag="sm", name="wps")
                        for kc in range(KC):
                            nc.tensor.matmul(
                                out=wps[:],
                                lhsT=outT[:, (kc * BL + b) * T + tlo:
                                          (kc * BL + b) * T + tlo + TB],
                                rhs=wsT[:, kc * D:(kc + 1) * D],
                                start=(kc == 0), stop=(kc == KC - 1))
                        if b == 0:
                            nc.scalar.copy(out=wstT_sb[:, b * D:(b + 1) * D],
                                           in_=wps[:])
                        else:
                            nc.vector.tensor_copy(
                                out=wstT_sb[:, b * D:(b + 1) * D], in_=wps[:])

                    atb = bp.tile([128, BL * TB], BF, tag="atb", name="atb")
                    ctxT = bp.tile([128, BL * KC * TB], BF, tag="ctxT",
                                   name="ctxT")
                    for b in range(BL):
                        blkstate[(blk, b)] = (atb, ctxT)
                        for half in range(2):
                            th_list = [None] * KC
                            cnt = [0]
                            for dc in range(KC):
                                pend_pe.append(
                                    (lambda blk=blk, b=b, half=half, dc=dc,
                                     w=wstT_sb, tl=th_list, cn=cnt:
                                     z_chunk(blk, b, half, dc, w, tl, cn)))

                while pend_pe or pend_act:
                    drain_pe(2)
                    drain_act(2)

                if DEBUG_TAPS:
                    nc.sync.dma_start(out=t_dbg_h[:], in_=outT[:])
                    nc.sync.dma_start(out=t_dbg_o2[:], in_=o2T[:])
                    nc.sync.dma_start(out=t_dbg_wh[:], in_=whsT[:])
                    nc.sync.dma_start(out=t_dbg_ws[0:16, :],
                                      in_=last_wst[0][:])
                    epssb = sp.tile([128, BL * T], F32)
                    nc.vector.tensor_copy(out=epssb[:], in_=eps2[:])
                    nc.sync.dma_start(out=t_dbg_e[:], in_=epssb[:])

            # ================= vocab projection =================
            # Vp_b is added on the host; output is bf16 (upcast on host).
            with tc.tile_pool(name="ps_v", bufs=2, space="PSUM") as ppv:
                for ib in range(NVB):
                    v0 = ib * VB
                    if ib < NPREB:
                        vsrc = vpre4[:, ib]
                    else:
                        vps = vp.tile([128, KC, VB], BF, tag="vps",
                                      name="vps")
                        nc.gpsimd.dma_start(out=vps[:], in_=vpt4[:, ib])
                        vsrc = vps
                    lsb = vp.tile([128, VB], BF, tag="lsb", bufs=3,
                                  name="lsb")
                    for sb in range(VB // VSUB):
                        s0 = sb * VSUB
                        lps = ppv.tile([128, VSUB], F32, tag="lps",
                                       name="lps")
                        for kc in range(KC):
                            nc.tensor.matmul(
                                out=lps[:],
                                lhsT=o2T[:, kc * 128:(kc + 1) * 128],
                                rhs=vsrc[:, kc, s0:s0 + VSUB],
                                start=(kc == 0), stop=(kc == KC - 1))
                        if sb % 2 == 0:
                            nc.scalar.copy(out=lsb[:, s0:s0 + VSUB],
                                           in_=lps[:])
                        else:
                            nc.vector.tensor_copy(out=lsb[:, s0:s0 + VSUB],
                                                  in_=lps[:])
                    nc.gpsimd.dma_start(out=t_out[:, v0:v0 + VB], in_=lsb[:])

    nc.compile()
    return nc


def _prep_in_maps(inputs):
    inp = {k: np.asarray(v) for k, v in inputs.items()}
    words = inp["words"].astype(np.int64)
    enc = inp["encoder_output"].astype(np.float32)
    pre_h, cell = inp["pre_h"], inp["cell"]
    emb = inp["emb"]
    W_ih, W_hh = inp["W_ih"], inp["W_hh"]
    b_ih, b_hh = inp["b_ih"], inp["b_hh"]
    Wh_w = inp["Wh_w"]
    Ws_w, Ws_b = inp["Ws_w"], inp["Ws_b"]
    vt_w = inp["vt_w"]
    V_w, V_b = inp["V_w"], inp["V_b"]
    Vp_w = inp["Vp_w"]

    P = np.array(GPERM)
    W_hh2 = W_hh.reshape(16, 128, D)[P].reshape(G4, D)
    W_ih2 = W_ih.reshape(16, 128, D)[P].reshape(G4, D)
    bias2 = (b_ih + b_hh).astype(np.float32).reshape(16, 128)[P].reshape(G4)

    def re_lhsT(m):  # [512, N] -> [128, 4*N] chunk-major, bf16
        n = m.shape[1]
        return np.ascontiguousarray(
            m.reshape(4, 128, n).transpose(1, 0, 2).reshape(128, 4 * n)
        ).astype(BF16)

    whh_re = re_lhsT(np.ascontiguousarray(W_hh2.T))
    whT_re = re_lhsT(np.ascontiguousarray(Wh_w.T))
    wsT_re = re_lhsT(np.ascontiguousarray(Ws_w.T))
    vw1_re = re_lhsT(np.ascontiguousarray(V_w[:, :D].T))
    vw2_re = re_lhsT(np.ascontiguousarray(V_w[:, D:].T))
    # [128,(kc,v)] -> [128,(nb,kc,vb)] so each vocab block is one contiguous
    # 16KB-per-partition DMA
    vpt_re = np.ascontiguousarray(
        re_lhsT(np.ascontiguousarray(Vp_w.T))
        .reshape(128, KC, NVB, VB).transpose(0, 2, 1, 3)
        .reshape(128, NVB * KC * VB))
    wsb_re = np.ascontiguousarray(Ws_b.reshape(4, 128).T).astype(np.float32)
    vb_re = np.ascontiguousarray(V_b.reshape(4, 128).T).astype(np.float32)
    vt_re = np.ascontiguousarray(vt_w.reshape(4, 128).T).astype(BF16)
    # selS[s', (t,s)] = (s == s')
    selS_re = np.zeros((S, TB * S), dtype=BF16)
    for tt in range(TB):
        selS_re[:, tt * S:(tt + 1) * S] = np.eye(S, dtype=BF16)
    # selT[t', (t,s)] = (t == t')
    selT_re = np.zeros((16, TB * S), dtype=BF16)
    for tt in range(TB):
        selT_re[tt, tt * S:(tt + 1) * S] = 1.0
    ident_re = np.eye(128, dtype=np.float32).astype(BF16)
    identF_re = np.eye(128, dtype=np.float32)

    x_all = emb[words]                                    # [B,T,D]
    xg_all = x_all @ W_ih2.T.astype(np.float32) + bias2   # [B,T,4D]

    in_maps = []
    for k in range(NC):
        bs = slice(k * BL, (k + 1) * BL)
        xgl = xg_all[bs]                                 # [2,T,2048]
        xg_re = np.ascontiguousarray(
            xgl.reshape(BL, T, 16, 128).transpose(3, 1, 2, 0)
            .reshape(128, T * 16 * BL)).astype(np.float32)
        h0 = np.ascontiguousarray(
            pre_h[bs].reshape(BL, 4, 128).transpose(2, 1, 0)
            .reshape(128, 4 * BL)).astype(BF16)
        c0 = np.ascontiguousarray(
            cell[bs].reshape(BL, 4, 128).transpose(2, 1, 0)
            .reshape(128, 4 * BL)).astype(np.float32)
        encl = enc[bs]                                   # [2,S,D]
        encT_re = np.ascontiguousarray(
            encl.reshape(BL, S, 4, 128).transpose(3, 2, 0, 1)
            .reshape(128, 4 * BL * S)).astype(BF16)
        enc_re = np.ascontiguousarray(
            encl.transpose(1, 0, 2).reshape(S, BL * D)).astype(BF16)
        in_maps.append({
            "xg": xg_re, "whh": whh_re, "h0": h0, "c0": c0,
            "encT": encT_re, "enc": enc_re, "whT": whT_re, "wsT": wsT_re,
            "vw1": vw1_re, "vw2": vw2_re, "wsb": wsb_re, "vb": vb_re,
            "vt": vt_re, "vpt": vpt_re, "selS": selS_re, "selT": selT_re,
            "ident": ident_re, "identF": identF_re,
        })
    return in_maps


def kernel(**inputs):
    in_maps = _prep_in_maps(inputs)
    if "nc" not in _cached:
        _cached["nc"] = _build_nc()
    res = bass_utils.run_bass_kernel_spmd(_cached["nc"], in_maps,
                                          core_ids=list(range(NC)))
    outs = [res.results[k]["out"].reshape(BL, T, V) for k in range(NC)]
    full = np.concatenate(outs, axis=0).astype(np.float32)
    full += np.asarray(inputs["Vp_b"]).astype(np.float32)
    return full


if __name__ == "__main__":
    pass


# revision 53
# speedup vs baseline: 1.6258x; 1.1489x over previous
"""AttnOutputDecoder Trainium2 kernel (v2).

Sharding: data-parallel over batch B=16 across 8 cores (2 batches/core).

Structure per core:
- LSTM with W-stationary bf16 matmuls; x@W_ih.T precomputed on host (f32)
  and folded into the gate PSUM via an identity-lhsT f32 matmul (no DVE add).
  Gate order (f,i,o,g) so one sigmoid covers f+i, then tanh(g), then
  sigmoid(o).
- Bahdanau scores via PE broadcast: z[d,(t,s)] = wh[s,d] + ws[t,d] built by
  two selector matmuls into PSUM, then one big amortized tanh (ACT) with
  Ws_b as the per-partition bias, then per-t dot with vt via tiny matmuls
  accumulating e[s,t] in PSUM. All interleaved with LSTM steps.
- Softmax / context / output-projection per 16-step block, pipelined.
- Full-vocab projection streamed in 2000-column blocks (Pool/SWDGE DMAs,
  16KB per partition each); output bf16; Vp_b added on host.
"""

import numpy as np
import ml_dtypes

import concourse.bass as bass
import concourse.mybir as mybir
import concourse.tile as tile
from concourse import bacc
from concourse import bass_utils

BF16 = ml_dtypes.bfloat16
F32 = mybir.dt.float32
BF = mybir.dt.bfloat16
AF = mybir.ActivationFunctionType
ALU = mybir.AluOpType

B, T, S, D, V = 16, 64, 128, 512, 32000
NC = 8
BL = B // NC          # local batches per core = 2
R = BL * T            # local rows = 128
G4 = 4 * D            # 2048 gates
KC = D // 128         # 4 contraction chunks
TB = 16               # t-block for the attention pipeline
NBLK = T // TB
VL = V // NC          # vocab shard per core = 4000 (tensor-parallel)
VSUB = 500            # matmul moving-dim sub-block
TH = T // 2           # gather half = 32 timesteps

# gate row-block permutation: pytorch (i,f,g,o) -> (f,i,o,g), 128-row blocks
GPERM = [4, 5, 6, 7, 0, 1, 2, 3, 12, 13, 14, 15, 8, 9, 10, 11]

_cached = {}
DEBUG_TAPS = False


def _build_nc():
    nc = bacc.Bacc("TRN2", target_bir_lowering=False, debug=False,
                   num_devices=NC)

    def din(name, shape, dt):
        return nc.dram_tensor(name, shape, dt, kind="ExternalInput").ap()

    t_xg = din("xg", [128, 16 * T * BL], F32)        # [p,(j,t,b)] perm order
    t_whh = din("whh", [128, KC * G4], BF)           # [p,(kc,g)] perm order
    t_h0 = din("h0", [128, KC * BL], BF)             # [p,(kc,b)]
    t_c0 = din("c0", [128, KC * BL], F32)
    t_encT = din("encT", [128, KC * BL * S], BF)     # [p,(kc,b,s)]
    t_enc = din("enc", [128, BL * D], BF)            # [s,(b,d)]
    t_whT = din("whT", [128, KC * D], BF)            # [p,(kc,d)] Wh_w.T re
    t_wsT = din("wsT", [128, KC * D], BF)
    t_vw1 = din("vw1", [128, KC * D], BF)            # (V_w[:,:D]).T re
    t_vw2 = din("vw2", [128, KC * D], BF)
    t_wsb = din("wsb", [128, KC], F32)               # Ws_b chunks
    t_vb = din("vb", [128, KC], F32)                 # V_b chunks
    t_vt = din("vt", [128, KC], BF)                  # vt_w chunks
    t_vpt = din("vpt", [128, KC * VL], BF)           # per-core vocab shard
    t_selS = din("selS", [128, 4 * S], BF)           # delta(s==s') per t
    t_selT = din("selT", [8, 8 * S], BF)             # delta(t==t') per s
    t_ident = din("ident", [128, 128], BF)
    t_identF = din("identF", [128, 128], F32)
    # rows: (half, corepair j, k in pair, b, t_local32) -> see host unscramble
    t_out = nc.dram_tensor("out", [NC * 128, VL], BF,
                           kind="ExternalOutput").ap()
    if DEBUG_TAPS:
        t_dbg_h = nc.dram_tensor("dbg_h", [128, KC * BL * T], BF,
                                 kind="ExternalOutput").ap()
        t_dbg_e = nc.dram_tensor("dbg_e", [128, BL * T], F32,
                                 kind="ExternalOutput").ap()
        t_dbg_o2 = nc.dram_tensor("dbg_o2", [128, KC * BL * T], BF,
                                  kind="ExternalOutput").ap()
        t_dbg_ws = nc.dram_tensor("dbg_ws", [128, BL * D], BF,
                                  kind="ExternalOutput").ap()
        t_dbg_wh = nc.dram_tensor("dbg_wh", [128, BL * D], BF,
                                  kind="ExternalOutput").ap()
        t_dbg_th = nc.dram_tensor("dbg_th", [128, 2 * KC * 8 * S], BF,
                                  kind="ExternalOutput").ap()

    with tile.TileContext(nc) as tc:
        with (
            tc.tile_pool(name="const", bufs=1) as cp,
            tc.tile_pool(name="state", bufs=1) as sp,
            tc.tile_pool(name="blk", bufs=2) as bp,
            tc.tile_pool(name="gates", bufs=2) as gp,
            tc.tile_pool(name="th", bufs=6) as thp,
            tc.tile_pool(name="attn", bufs=3) as ap_,
            tc.tile_pool(name="voc", bufs=2) as vp,
            tc.tile_pool(name="dram", bufs=1, space="DRAM") as dmp,
        ):
            # ---- resident constants (DMA order = LSTM-critical first) ----
            whh = cp.tile([128, KC * G4], BF)
            nc.sync.dma_start(out=whh[:], in_=t_whh[:])
            xg = cp.tile([128, 16 * T * BL], F32)
            xgD = t_xg[:].rearrange("p (t j b) -> p t j b", j=16, t=T, b=BL)
            xg4 = xg[:].rearrange("p (t j b) -> p t j b", j=16, t=T, b=BL)
            nc.sync.dma_start(out=xg4[:, 0:TB], in_=xgD[:, 0:TB])
            h0bf = sp.tile([128, KC * BL], BF)
            nc.sync.dma_start(out=h0bf[:], in_=t_h0[:])
            c = sp.tile([128, KC * BL], F32)
            nc.sync.dma_start(out=c[:], in_=t_c0[:])
            identF = cp.tile([128, 128], F32)
            nc.sync.dma_start(out=identF[:], in_=t_identF[:])
            wsb = cp.tile([128, KC], F32)
            nc.sync.dma_start(out=wsb[:], in_=t_wsb[:])
            vb = cp.tile([128, KC], F32)
            nc.sync.dma_start(out=vb[:], in_=t_vb[:])
            vt = cp.tile([128, KC], BF)
            nc.sync.dma_start(out=vt[:], in_=t_vt[:])
            for tchunk in range(1, 4):
                nc.sync.dma_start(out=xg4[:, tchunk * TB:(tchunk + 1) * TB],
                                  in_=xgD[:, tchunk * TB:(tchunk + 1) * TB])
            encT = cp.tile([128, KC * BL * S], BF)
            nc.sync.dma_start(out=encT[:], in_=t_encT[:])
            whT = cp.tile([128, KC * D], BF)
            nc.sync.dma_start(out=whT[:], in_=t_whT[:])
            wsT = cp.tile([128, KC * D], BF)
            nc.sync.dma_start(out=wsT[:], in_=t_wsT[:])
            selS = cp.tile([128, 4 * S], BF)
            nc.sync.dma_start(out=selS[:], in_=t_selS[:])
            selT = cp.tile([8, 8 * S], BF)
            nc.sync.dma_start(out=selT[:], in_=t_selT[:])
            enc = cp.tile([128, BL * D], BF)
            nc.sync.dma_start(out=enc[:], in_=t_enc[:])
            vw1 = cp.tile([128, KC * D], BF)
            nc.sync.dma_start(out=vw1[:], in_=t_vw1[:])
            vw2 = cp.tile([128, KC * D], BF)
            nc.sync.dma_start(out=vw2[:], in_=t_vw2[:])
            ident = cp.tile([128, 128], BF)
            nc.sync.dma_start(out=ident[:], in_=t_ident[:])

            # ---- state ----
            outT = sp.tile([128, KC * BL * T], BF)   # [p,(kc,b,t)] all h's
            o2T = sp.tile([128, KC * BL * T], BF)    # [p,(ec,b,t)]
            whsT = sp.tile([128, BL * D], BF)        # [s,(b,d)] wh transposed

            h0bf3 = h0bf[:].rearrange("p (kc b) -> p kc b", kc=KC, b=BL)
            outT4 = outT[:].rearrange("p (kc b t) -> p kc b t", kc=KC, b=BL,
                                      t=T)
            o2T3 = o2T[:].rearrange("p (ec b t) -> p (ec b) t", ec=KC, b=BL)

            # ---- resident vocab shard weights (Pool/SWDGE DMAs) ----
            vres = cp.tile([128, KC * VL], BF)
            vres3 = vres[:].rearrange("p (kc v) -> p kc v", kc=KC, v=VL)
            vptD = t_vpt[:].rearrange("p (kc v) -> p kc v", kc=KC, v=VL)
            for kc in range(KC):
                nc.gpsimd.dma_start(out=vres3[:, kc], in_=vptD[:, kc])

            # ---- all-gather staging: o2 for all 16 batches ----
            # o2g[p=dchunk-row, (half, k, ec, b, t32)]
            o2g = sp.tile([128, 2 * NC * KC * BL * TH], BF)
            # row-tile rows (kk,b,t) contiguous per (half, corepair, dchunk)
            o2g7 = o2g[:].rearrange("p (h j ec kk b t) -> p h j ec kk b t",
                                    h=2, j=NC // 2, ec=KC, kk=2, b=BL, t=TH)
            o2T4 = o2T[:].rearrange("p (ec b t) -> p ec b t", ec=KC, b=BL,
                                    t=T)

            with (
                tc.tile_pool(name="ps_g", bufs=1, space="PSUM") as ppg,
                tc.tile_pool(name="ps_z", bufs=2, space="PSUM") as ppz,
                tc.tile_pool(name="ps_e", bufs=1, space="PSUM") as ppe,
                tc.tile_pool(name="ps_sm", bufs=2, space="PSUM") as pps,
            ):
                eps2 = ppe.tile([128, BL * T], F32)  # e[s,(b,t)]
                eps3 = eps2[:].rearrange("p (b t) -> p b t", b=BL, t=T)

                # ---------- pipelined closures (engine-split queues) -------
                pend_pe = []
                pend_act = []

                def whsT_calc(b):
                    # whsT = (enc @ Wh_w.T) in [s,(b,d)] layout
                    wps = pps.tile([128, 512], F32, tag="sm", name="wps")
                    for kc in range(KC):
                        nc.tensor.matmul(
                            out=wps[:],
                            lhsT=encT[:, (kc * BL + b) * S:(kc * BL + b + 1) * S],
                            rhs=whT[:, kc * D:(kc + 1) * D],
                            start=(kc == 0), stop=(kc == KC - 1))
                    nc.vector.tensor_copy(out=whsT[:, b * D:(b + 1) * D],
                                          in_=wps[:])

                def z_chunk(blk, b, half, dc, wstT_sb, th_list, cnt):
                    # z[d,(t8,s128)] = wh[s,d]+ws[t,d] (PE broadcast matmuls)
                    zt = ppz.tile([128, 8 * S], F32, tag="zt", name="zt")
                    for g in range(2):
                        zo = g * 4 * S
                        nc.tensor.matmul(
                            out=zt[:, zo:zo + 4 * S],
                            lhsT=whsT[:, b * D + dc * 128:
                                      b * D + (dc + 1) * 128],
                            rhs=selS[:],
                            start=True, stop=False)
                        nc.tensor.matmul(
                            out=zt[:, zo:zo + 4 * S],
                            lhsT=wstT_sb[:, b * D + dc * 128:
                                         b * D + (dc + 1) * 128],
                            rhs=selT[:, zo:zo + 4 * S],
                            start=False, stop=True)
                    pend_act.append(
                        lambda: z_tanh(blk, b, half, dc, zt, th_list, cnt))

                def z_tanh(blk, b, half, dc, zt, th_list, cnt):
                    th = thp.tile([128, 8 * S], BF, tag="th", name="th")
                    nc.scalar.activation(out=th[:], in_=zt[:], func=AF.Tanh,
                                         bias=wsb[:, dc:dc + 1])
                    if DEBUG_TAPS and blk == 0 and b == 0:
                        off = (half * KC + dc) * 8 * S
                        nc.sync.dma_start(
                            out=t_dbg_th[:, off:off + 8 * S], in_=th[:])
                    th_list[dc] = th
                    cnt[0] += 1
                    if cnt[0] == KC:
                        pend_pe.append(lambda: dots(blk, b, half, th_list))

                def dots(blk, b, half, th_list):
                    # e[s,t] = sum_d th[d,(t,s)]*vt[d]; one complete
                    # 4-chunk PSUM group per t (PSUM zero-regions forbid
                    # interleaved groups within a bank)
                    tlo = blk * TB
                    for tl in range(8):
                        t = tlo + half * 8 + tl
                        for dc in range(KC):
                            nc.tensor.matmul(
                                out=eps3[:, b, t:t + 1],
                                lhsT=th_list[dc][:, tl * S:(tl + 1) * S],
                                rhs=vt[:, dc:dc + 1],
                                start=(dc == 0), stop=(dc == KC - 1))
                    if half == 1:
                        ab, cx = blkstate[(blk, b)]
                        pend_act.append(lambda: softmax(blk, b, ab, cx))

                def softmax(blk, b, atb, ctxT):
                    # exp(x) = (1+u)/(1-u), u = tanh(x/2): stays in the
                    # sigmoid/tanh ACT table (no act-table reload)
                    tlo = blk * TB
                    ebf = ap_.tile([128, TB], BF, tag="ebf", name="ebf")
                    nc.scalar.activation(out=ebf[:],
                                         in_=eps3[:, b, tlo:tlo + TB],
                                         func=AF.Tanh, scale=0.5)
                    etp = pps.tile([TB, S], BF, tag="sm", name="etp")
                    nc.tensor.transpose(out=etp[:], in_=ebf[:],
                                        identity=ident[:, :])
                    # num = 1+u ; den = 1-u ; e^x = num/den
                    num = ap_.tile([TB, S], F32, tag="num", name="num")
                    nc.vector.tensor_scalar(out=num[:], in0=etp[:],
                                            scalar1=1.0, scalar2=None,
                                            op0=ALU.add)
                    den = ap_.tile([TB, S], F32, tag="den", name="den")
                    nc.vector.tensor_scalar(out=den[:], in0=etp[:],
                                            scalar1=-1.0, scalar2=1.0,
                                            op0=ALU.mult, op1=ALU.add)
                    rden = ap_.tile([TB, S], F32, tag="rden", name="rden")
                    nc.vector.reciprocal(out=rden[:], in_=den[:])
                    ex = ap_.tile([TB, S], F32, tag="ex", name="ex")
                    nc.vector.tensor_mul(out=ex[:], in0=num[:], in1=rden[:])
                    ssum = ap_.tile([TB, 1], F32, tag="ssum", name="ssum")
                    nc.vector.tensor_reduce(out=ssum[:], in_=ex[:],
                                            axis=mybir.AxisListType.X,
                                            op=ALU.add)
                    rsum = ap_.tile([TB, 1], F32, tag="rsum", name="rsum")
                    nc.vector.reciprocal(out=rsum[:], in_=ssum[:])
                    abf = ap_.tile([TB, S], BF, tag="abf", name="abf")
                    nc.vector.tensor_scalar_mul(out=abf[:], in0=ex[:],
                                                scalar1=rsum[:])
                    atp = pps.tile([S, TB], BF, tag="sm", name="atp")
                    nc.tensor.transpose(out=atp[:], in_=abf[:],
                                        identity=ident[0:TB, 0:TB])
                    nc.vector.tensor_copy(out=atb[:, b * TB:(b + 1) * TB],
                                          in_=atp[:])
                    for dc in range(KC):
                        pend_pe.append(
                            (lambda dc=dc: ctx(blk, b, dc, atb, ctxT)))

                def ctx(blk, b, dc, atb, ctxT):
                    cps = pps.tile([128, TB], F32, tag="sm", name="cps")
                    nc.tensor.matmul(out=cps[:],
                                     lhsT=enc[:, b * D + dc * 128:
                                              b * D + (dc + 1) * 128],
                                     rhs=atb[:, b * TB:(b + 1) * TB],
                                     start=True, stop=True)
                    if (dc + b) % 2 == 0:
                        nc.scalar.copy(
                            out=ctxT[:, (b * KC + dc) * TB:
                                     (b * KC + dc + 1) * TB], in_=cps[:])
                    else:
                        nc.vector.tensor_copy(
                            out=ctxT[:, (b * KC + dc) * TB:
                                     (b * KC + dc + 1) * TB], in_=cps[:])
                    if dc == KC - 1:
                        for ec in range(KC):
                            pend_pe.append(
                                (lambda ec=ec: o2(blk, b, ec, ctxT)))

                def o2(blk, b, ec, ctxT):
                    tlo = blk * TB
                    ops = pps.tile([128, TB], F32, tag="sm", name="ops")
                    for kc in range(KC):
                        nc.tensor.matmul(
                            out=ops[:],
                            lhsT=vw1[:, kc * D + ec * 128:
                                     kc * D + (ec + 1) * 128],
                            rhs=ctxT[:, (b * KC + kc) * TB:
                                     (b * KC + kc + 1) * TB],
                            start=(kc == 0), stop=False)
                    for kc in range(KC):
                        nc.tensor.matmul(
                            out=ops[:],
                            lhsT=vw2[:, kc * D + ec * 128:
                                     kc * D + (ec + 1) * 128],
                            rhs=outT[:, (kc * BL + b) * T + tlo:
                                     (kc * BL + b) * T + tlo + TB],
                            start=False, stop=(kc == KC - 1))
                    nc.vector.tensor_scalar(
                        out=o2T[:, ec * R + b * T + tlo:
                                ec * R + b * T + tlo + TB],
                        in0=ops[:], scalar1=vb[:, ec:ec + 1], scalar2=None,
                        op0=ALU.add)
                    h = blk // 2
                    o2done[h] += 1
                    if o2done[h] == 2 * BL * KC:
                        # all o2 writes for this t-half emitted -> the
                        # gather's DMA deps are now complete
                        pend_pe.append(lambda h=h: gather(h))

                blkstate = {}
                last_wst = [None]
                o2done = [0, 0]

                def drain_pe(n):
                    for _ in range(min(n, len(pend_pe))):
                        pend_pe.pop(0)()

                def drain_act(n):
                    for _ in range(min(n, len(pend_act))):
                        pend_act.pop(0)()

                def gather(h):
                    # all-gather o2 for t-half h across the 8 cores
                    bin_ = dmp.tile([128, KC * BL * TH], BF, name="bin",
                                    tag=f"bin{h}")
                    bout = dmp.tile([NC * 128, KC * BL * TH], BF,
                                    name="bout", tag=f"bout{h}")
                    nc.gpsimd.dma_start(
                        out=bin_[:],
                        in_=o2T4[:, :, :, h * TH:(h + 1) * TH])
                    nc.gpsimd.collective_compute(
                        "AllGather",
                        mybir.AluOpType.bypass,
                        replica_groups=[list(range(NC))],
                        ins=[bin_.opt()],
                        outs=[bout.opt()],
                    )
                    for k in range(NC):
                        nc.gpsimd.dma_start(
                            out=o2g7[:, h, k // 2, :, k % 2],
                            in_=bout[k * 128:(k + 1) * 128, :].rearrange(
                                "p (ec b t) -> p ec b t", ec=KC, b=BL, t=TH))

                # ================== LSTM + pipeline ==================
                for blk in range(NBLK):
                    tlo = blk * TB
                    for t in range(tlo, tlo + TB):
                        gps = ppg.tile([128, 32], F32, tag="gps", name="gps")
                        for j in range(16):
                            for kc in range(KC):
                                rhs = (h0bf3[:, kc, :] if t == 0
                                       else outT4[:, kc, :, t - 1])
                                nc.tensor.matmul(
                                    out=gps[:, 2 * j:2 * j + 2],
                                    lhsT=whh[:, kc * G4 + j * 128:
                                              kc * G4 + (j + 1) * 128],
                                    rhs=rhs, start=(kc == 0), stop=False)
                            nc.tensor.matmul(out=gps[:, 2 * j:2 * j + 2],
                                             lhsT=identF[:],
                                             rhs=xg4[:, t, j, :],
                                             start=False, stop=True)
                        # PE-side pipeline work queues behind the (chain-
                        # critical) gate matmuls, ahead of next step's gates
                        drain_pe(3)
                        sio = gp.tile([128, 24], F32, tag="sio", name="sio")
                        nc.scalar.activation(out=sio[:, 0:24],
                                             in_=gps[:, 0:24],
                                             func=AF.Sigmoid)
                        tg = gp.tile([128, 8], F32, tag="tg", name="tg")
                        nc.scalar.activation(out=tg[:], in_=gps[:, 24:32],
                                             func=AF.Tanh)
                        t1 = gp.tile([128, 8], F32, tag="t1", name="t1")
                        t2 = gp.tile([128, 8], F32, tag="t2", name="t2")
                        nc.vector.tensor_mul(out=t1[:], in0=sio[:, 0:8],
                                             in1=c[:])
                        nc.vector.tensor_mul(out=t2[:], in0=sio[:, 8:16],
                                             in1=tg[:])
                        nc.vector.tensor_add(out=c[:], in0=t1[:], in1=t2[:])
                        tc_ = gp.tile([128, 8], F32, tag="tc", name="tc")
                        nc.scalar.activation(out=tc_[:], in_=c[:],
                                             func=AF.Tanh)
                        sio3 = sio[:, 16:24].rearrange(
                            "p (kc b) -> p kc b", kc=KC, b=BL)
                        tc3 = tc_[:].rearrange("p (kc b) -> p kc b",
                                               kc=KC, b=BL)
                        nc.vector.tensor_mul(out=outT4[:, :, :, t],
                                             in0=sio3, in1=tc3)
                        # ACT-side pipeline work after the step's ACT chain
                        drain_act(2 if t % 2 == 0 else 1)

                        if (t - tlo) % 8 == 7:
                            hf = (t - tlo) // 8
                            if blk == 0 and hf == 0:
                                for b in range(BL):
                                    pend_pe.append(
                                        lambda b=b: whsT_calc(b))
                            # ws for this 8-step half, transposed [t8,(b,d)]
                            wstT_sb = bp.tile([8, BL * D], BF, tag="wstT",
                                              bufs=3, name="wstT")
                            last_wst[0] = wstT_sb
                            t0 = tlo + hf * 8
                            for b in range(BL):
                                wps = pps.tile([8, 512], F32, tag="sm",
                                               name="wps")
                                for kc in range(KC):
                                    nc.tensor.matmul(
                                        out=wps[:],
                                        lhsT=outT[:, (kc * BL + b) * T + t0:
                                                  (kc * BL + b) * T + t0 + 8],
                                        rhs=wsT[:, kc * D:(kc + 1) * D],
                                        start=(kc == 0), stop=(kc == KC - 1))
                                if b == 0:
                                    nc.scalar.copy(
                                        out=wstT_sb[:, b * D:(b + 1) * D],
                                        in_=wps[:])
                                else:
                                    nc.vector.tensor_copy(
                                        out=wstT_sb[:, b * D:(b + 1) * D],
                                        in_=wps[:])
                            if hf == 0:
                                atb = bp.tile([128, BL * TB], BF, tag="atb",
                                              name="atb")
                                ctxT = bp.tile([128, BL * KC * TB], BF,
                                               tag="ctxT", name="ctxT")
                                for b in range(BL):
                                    blkstate[(blk, b)] = (atb, ctxT)
                            for b in range(BL):
                                th_list = [None] * KC
                                cnt = [0]
                                for dc in range(KC):
                                    pend_pe.append(
                                        (lambda blk=blk, b=b, half=hf, dc=dc,
                                         w=wstT_sb, tl=th_list, cn=cnt:
                                         z_chunk(blk, b, half, dc, w, tl,
                                                 cn)))

                while pend_pe or pend_act:
                    drain_pe(2)
                    drain_act(2)

                if DEBUG_TAPS:
                    nc.sync.dma_start(out=t_dbg_h[:], in_=outT[:])
                    nc.sync.dma_start(out=t_dbg_o2[:], in_=o2T[:])
                    nc.sync.dma_start(out=t_dbg_wh[:], in_=whsT[:])
                    nc.sync.dma_start(out=t_dbg_ws[0:8, :],
                                      in_=last_wst[0][:])
                    epssb = sp.tile([128, BL * T], F32)
                    nc.vector.tensor_copy(out=epssb[:], in_=eps2[:])
                    nc.sync.dma_start(out=t_dbg_e[:], in_=epssb[:])

            # ================= vocab projection (sharded) =================
            # Each core: all 16 batches x its 4000-col vocab slice.
            # Row tile = (core-pair k2, b2, t32) for gather-half h.
            # Vp_b is added on the host; output is bf16 (upcast on host).
            with tc.tile_pool(name="ps_v", bufs=2, space="PSUM") as ppv:
                for h in range(2):
                    for j in range(4):
                        lsb = vp.tile([128, VL], BF, tag="lsb", bufs=3,
                                      name="lsb")
                        for sb in range(VL // VSUB):
                            s0 = sb * VSUB
                            lps = ppv.tile([128, VSUB], F32, tag="lps",
                                           name="lps")
                            for kc in range(KC):
                                nc.tensor.matmul(
                                    out=lps[:],
                                    lhsT=o2g7[:, h, j, kc],
                                    rhs=vres3[:, kc, s0:s0 + VSUB],
                                    start=(kc == 0), stop=(kc == KC - 1))
                            if sb % 2 == 0:
                                nc.scalar.copy(out=lsb[:, s0:s0 + VSUB],
                                               in_=lps[:])
                            else:
                                nc.vector.tensor_copy(
                                    out=lsb[:, s0:s0 + VSUB], in_=lps[:])
                        nc.gpsimd.dma_start(
                            out=t_out[(h * 4 + j) * 128:
                                      (h * 4 + j + 1) * 128, :],
                            in_=lsb[:])

    nc.compile()
    return nc


def _prep_in_maps(inputs):
    inp = {k: np.asarray(v) for k, v in inputs.items()}
    words = inp["words"].astype(np.int64)
    enc = inp["encoder_output"].astype(np.float32)
    pre_h, cell = inp["pre_h"], inp["cell"]
    emb = inp["emb"]
    W_ih, W_hh = inp["W_ih"], inp["W_hh"]
    b_ih, b_hh = inp["b_ih"], inp["b_hh"]
    Wh_w = inp["Wh_w"]
    Ws_w, Ws_b = inp["Ws_w"], inp["Ws_b"]
    vt_w = inp["vt_w"]
    V_w, V_b = inp["V_w"], inp["V_b"]
    Vp_w = inp["Vp_w"]

    P = np.array(GPERM)
    W_hh2 = W_hh.reshape(16, 128, D)[P].reshape(G4, D)
    W_ih2 = W_ih.reshape(16, 128, D)[P].reshape(G4, D)
    bias2 = (b_ih + b_hh).astype(np.float32).reshape(16, 128)[P].reshape(G4)

    def re_lhsT(m):  # [512, N] -> [128, 4*N] chunk-major, bf16
        n = m.shape[1]
        return np.ascontiguousarray(
            m.reshape(4, 128, n).transpose(1, 0, 2).reshape(128, 4 * n)
        ).astype(BF16)

    whh_re = re_lhsT(np.ascontiguousarray(W_hh2.T))
    whT_re = re_lhsT(np.ascontiguousarray(Wh_w.T))
    wsT_re = re_lhsT(np.ascontiguousarray(Ws_w.T))
    vw1_re = re_lhsT(np.ascontiguousarray(V_w[:, :D].T))
    vw2_re = re_lhsT(np.ascontiguousarray(V_w[:, D:].T))
    wsb_re = np.ascontiguousarray(Ws_b.reshape(4, 128).T).astype(np.float32)
    vb_re = np.ascontiguousarray(V_b.reshape(4, 128).T).astype(np.float32)
    vt_re = np.ascontiguousarray(vt_w.reshape(4, 128).T).astype(BF16)
    # selS[s', (t4,s)] = (s == s')  (same pattern for every t)
    selS_re = np.zeros((S, 4 * S), dtype=BF16)
    for tt in range(4):
        selS_re[:, tt * S:(tt + 1) * S] = np.eye(S, dtype=BF16)
    # selT[t', (t8,s)] = (t == t')  (t' local within the 8-step half)
    selT_re = np.zeros((8, 8 * S), dtype=BF16)
    for tt in range(8):
        selT_re[tt, tt * S:(tt + 1) * S] = 1.0
    ident_re = np.eye(128, dtype=np.float32).astype(BF16)
    identF_re = np.eye(128, dtype=np.float32)

    x_all = emb[words]                                    # [B,T,D]
    xg_all = x_all @ W_ih2.T.astype(np.float32) + bias2   # [B,T,4D]

    in_maps = []
    for k in range(NC):
        bs = slice(k * BL, (k + 1) * BL)
        xgl = xg_all[bs]                                 # [2,T,2048]
        xg_re = np.ascontiguousarray(
            xgl.reshape(BL, T, 16, 128).transpose(3, 1, 2, 0)
            .reshape(128, T * 16 * BL)).astype(np.float32)
        h0 = np.ascontiguousarray(
            pre_h[bs].reshape(BL, 4, 128).transpose(2, 1, 0)
            .reshape(128, 4 * BL)).astype(BF16)
        c0 = np.ascontiguousarray(
            cell[bs].reshape(BL, 4, 128).transpose(2, 1, 0)
            .reshape(128, 4 * BL)).astype(np.float32)
        encl = enc[bs]                                   # [2,S,D]
        encT_re = np.ascontiguousarray(
            encl.reshape(BL, S, 4, 128).transpose(3, 2, 0, 1)
            .reshape(128, 4 * BL * S)).astype(BF16)
        enc_re = np.ascontiguousarray(
            encl.transpose(1, 0, 2).reshape(S, BL * D)).astype(BF16)
        vpt_re = re_lhsT(
            np.ascontiguousarray(Vp_w[k * VL:(k + 1) * VL].T))
        in_maps.append({
            "xg": xg_re, "whh": whh_re, "h0": h0, "c0": c0,
            "encT": encT_re, "enc": enc_re, "whT": whT_re, "wsT": wsT_re,
            "vw1": vw1_re, "vw2": vw2_re, "wsb": wsb_re, "vb": vb_re,
            "vt": vt_re, "vpt": vpt_re, "selS": selS_re, "selT": selT_re,
            "ident": ident_re, "identF": identF_re,
        })
    return in_maps


def kernel(**inputs):
    in_maps = _prep_in_maps(inputs)
    if "nc" not in _cached:
        _cached["nc"] = _build_nc()
    res = bass_utils.run_bass_kernel_spmd(_cached["nc"], in_maps,
                                          core_ids=list(range(NC)))
    full = np.empty((B, T, V), dtype=np.float32)
    for k in range(NC):
        # rows: (h, j, kk, b, tl) -> batch = 4j+2kk+b, t = h*32+tl
        oc = res.results[k]["out"].astype(np.float32)
        oc = oc.reshape(2, 4, BL, BL, TH, VL).transpose(1, 2, 3, 0, 4, 5)
        full[:, :, k * VL:(k + 1) * VL] = oc.reshape(B, T, VL)
    full += np.asarray(inputs["Vp_b"]).astype(np.float32)
    return full


if __name__ == "__main__":
    pass
